# revision 14
# baseline (speedup 1.0000x reference)
"""Channel-attention (CAM) Trainium2 Bass kernel.

Reference computation (per batch n):
    v = x[n].reshape(C, S)                 # C=512, S=H*W=4096
    energy = v @ v.T                       # (C, C)
    att = softmax(max_row(energy) - energy, axis=-1)
        = exp(min_row(energy) - energy) / Z
    out[n] = gamma * (att @ v) + x[n]

Sharding: data-parallel over N=8 batches across 8 NeuronCores; each core
computes one full C x C attention locally (no collectives).

v3 design (baseline 61377ns -> target ~41us). Two structural changes:

1. BF16 output (rel-err ~2e-3 vs the 2e-2 gate) halves store DMA
   23.3us -> 11.65us, which makes the drain ENGINE-bound: every einsum2
   group must cross PSUM->SBUF on DVE or ACT (gpsimd has no PSUM port,
   DMA cannot read PSUM, PE cannot read PSUM).

2. einsum2 groups accumulate in BF16 PSUM, 1024 wide. Consequences:
   - a group is ONE PSUM bank, so the shared staging tag rotates 4 deep
     (vs 2 with f32 groups) and the fill->drain->refill cycle pipelines;
   - DVE epilogues hit the 2x_1p fast mode (all operands 2-byte):
     ~818ns per 1024 instead of 1352;
   - precision: with gamma=0 the group holds exactly bf16(x) (identity
     matmul of bf16 x, no rounding loss); nonzero gamma adds bf16
     rounding of the attention part, well inside the 2e-1 regime.

   Route 'D' groups: one DVE tensor_add (bf16 PSUM + vb -> vb, 2x mode).
   Route 'A' groups: PE identity-matmul folds +x into PSUM (~245ns),
   then one bare ACT copy PSUM -> vb.

Other structure:
   - the energy softmax prologues (row-min DVE, exp ACT with Z-accum,
     dsc = gamma*I/Z) are emitted two rows AHEAD of the einsum2 drain so
     both engines always have independent queued work;
   - all einsum1 tail chunks run row-0-only inline during the loads
     (KPE chunks run all 4 rows), so row 0 closes right after the last
     load; rows 1-3 finish in phase 2 (PE, overlapped with softmax 0);
   - PSUM->vbT transpose staging moves in PAIRS of chunks (one
     1024-wide copy per two chunks) alternating DVE/ACT;
   - gpsimd carries the fp8 casts of v (einsum2 rhs), a few on ACT;
   - PSUM: 4 banks energy + 4 rotating 2KB slots shared by transpose
     pairs, bf16 einsum2 groups and the PT staging tile.
"""

import os
from contextlib import ExitStack

import numpy as np

import concourse.bass as bass
import concourse.tile as tile
from concourse import bacc, mybir
from concourse.bass_utils import run_bass_kernel_spmd
from concourse.masks import make_identity

N_CORES = 8
C = 512
S = 4096
P = 128
CI = C // P  # 4 c-chunks
KD = S // P  # 32 s-chunks of 128
OW = 1024    # einsum2 output group width (1 bf16 PSUM bank)
OG = S // OW  # 4 groups per row

STRIPES = [512] * 7 + [384, 128]
KPE = 22        # chunks whose einsum1 runs all 4 rows inline during loads
TAILF32 = 6     # trailing chunks transposed from xf in f32 (no vb cast dep)
# einsum2 route per (row, group): 'D' = DVE add (PSUM+xf->vb), 'A' = PE
# identity-add + ACT copy. g3 must be 'D': its columns overlap the TAILF32
# region, which never gets a bf16 cast into vb.
ROUTES = (
    ("A", "D", "A", "D"),
    ("A", "D", "A", "D"),
    ("A", "D", "A", "D"),
    ("A", "A", "D", "D"),
)
# engine for each vbT pair copy: 'D' (DVE) / 'A' (ACT)
PAIR_ENG = list("DADADADADADADADA")
# engine for each of the 36 (stripe, ci) fp8 casts: gpsimd with some ACT
VB8_ENG = list("GGGGGGGGGGGGGGGGAGGGAGGGAGGGAGGGAGGG")

F32 = mybir.dt.float32
BF16 = mybir.dt.bfloat16
FP8 = mybir.dt.float8e4


def _body(ctx: ExitStack, tc: tile.TileContext, out: bass.AP, x: bass.AP,
          gamma: bass.AP):
    nc = tc.nc

    persist = ctx.enter_context(tc.tile_pool(name="persist", bufs=1))
    xf = persist.tile([P, CI, S], F32, name="xf")
    vb = persist.tile([P, CI, S], BF16, name="vb")   # bf16 x, then output
    vb8 = persist.tile([P, CI, S], FP8, name="vb8")
    vbT = persist.tile([P, KD, C], BF16, name="vbT")
    p_sb = persist.tile([P, CI, C], BF16, name="p_sb")
    pt_sb = persist.tile([P, CI, C], FP8, name="pt_sb")
    ident = persist.tile([P, P], BF16, name="ident")
    identf = persist.tile([P, P], F32, name="identf")
    identg = persist.tile([P, P], BF16, name="identg")
    gamma_sb = persist.tile([P, 1], F32, name="gamma_sb")
    mn = persist.tile([P, CI], F32, name="mn")
    zsum = persist.tile([P, CI], F32, name="zsum")
    msc = persist.tile([P, CI], F32, name="msc")

    make_identity(nc, ident)
    make_identity(nc, identf)

    x3 = x.rearrange("(ci p) s -> p ci s", p=P)
    out3 = out.rearrange("(ci p) s -> p ci s", p=P)

    epool = ctx.enter_context(tc.tile_pool(name="epool", bufs=4, space="PSUM"))
    e_ps = [epool.tile([P, C], F32, name=f"e{ci}", tag="et") for ci in range(CI)]
    # 2 rotating 4KB slots shared (by tag) between transpose staging pairs,
    # f32 einsum2 groups and the PT staging tile.
    opool = ctx.enter_context(tc.tile_pool(name="opool", bufs=2, space="PSUM"))
    trpool = ctx.enter_context(tc.tile_pool(name="trp", bufs=3))
    dscpool = ctx.enter_context(tc.tile_pool(name="dscp", bufs=2))

    # ---- load + cast + PE-transpose + inline einsum1 ----
    def emit_e1(k):
        rows = range(CI) if k < KPE else (0,)
        for ci in rows:
            nc.tensor.matmul(
                e_ps[ci][:, ci * P:],
                lhsT=vbT[:, k, ci * P:(ci + 1) * P],
                rhs=vbT[:, k, ci * P:],
                start=(k == 0),
                stop=(k == KD - 1),
            )

    tp_cur = None
    col = 0
    for si, w in enumerate(STRIPES):
        sl = slice(col, col + w)
        for ci in range(CI):
            nc.sync.dma_start(out=xf[:, ci, sl], in_=x3[:, ci, sl])
            cast_hi = min(col + w, (KD - TAILF32) * P)
            if col < cast_hi:
                with tc.high_priority():
                    nc.vector.tensor_copy(out=vb[:, ci, col:cast_hi],
                                          in_=xf[:, ci, col:cast_hi])
            eng = VB8_ENG[si * CI + ci]
            if eng == "A":
                nc.scalar.copy(out=vb8[:, ci, sl], in_=xf[:, ci, sl])
            else:
                nc.gpsimd.tensor_copy(out=vb8[:, ci, sl], in_=xf[:, ci, sl])
        if si == 0:
            nc.sync.dma_start(out=gamma_sb[:, :],
                              in_=gamma.to_broadcast((P, 1)))
            # gamma*I once; dsc = identg * (1/Z) per row
            nc.vector.tensor_scalar(
                out=identg[:, :], in0=ident[:, :], scalar1=gamma_sb[:, :],
                scalar2=None, op0=mybir.AluOpType.mult,
            )
        for k in range(col // P, (col + w) // P):
            tailf = k >= KD - TAILF32
            j, half = k // 2, k % 2
            if tailf:
                # single-chunk staging: 2KB tiles ping-pong through the
                # 2-slot rotation at ~700ns/chunk instead of ~1.9us/pair
                tp_cur = opool.tile([P, C], F32, name="tps", tag="op")
                for ci in range(CI):
                    nc.tensor.transpose(
                        out=tp_cur[:, ci * P:(ci + 1) * P],
                        in_=xf[:, ci, k * P:(k + 1) * P],
                        identity=identf[:, :],
                    )
                if k % 2 == 0:
                    nc.scalar.copy(out=vbT[:, k, :], in_=tp_cur[:, :])
                else:
                    nc.vector.tensor_copy(out=vbT[:, k, :], in_=tp_cur[:, :])
                emit_e1(k - 2)
                continue
            if half == 0:
                tp_cur = opool.tile([P, 2, C], F32 if tailf else BF16,
                                    name="tp", tag="op")
            for ci in range(CI):
                nc.tensor.transpose(
                    out=tp_cur[:, half, ci * P:(ci + 1) * P],
                    in_=(xf if tailf else vb)[:, ci, k * P:(k + 1) * P],
                    identity=(identf if tailf else ident)[:, :],
                )
            if half == 1:
                # one 1024-wide PSUM->SBUF copy moves the whole pair
                if PAIR_ENG[j] == "D":
                    nc.vector.tensor_copy(out=vbT[:, 2 * j:2 * j + 2, :],
                                          in_=tp_cur[:, :, :])
                else:
                    nc.scalar.copy(out=vbT[:, 2 * j:2 * j + 2, :],
                                   in_=tp_cur[:, :, :])
                # einsum1 for the PREVIOUS pair (its copy landed during
                # this pair's transposes -- the in-order PE never stalls
                # on a copy it just triggered)
                if j > 0:
                    emit_e1(2 * (j - 1))
                    emit_e1(2 * (j - 1) + 1)
        col += w
    emit_e1(KD - 2)
    emit_e1(KD - 1)

    # ---- phase 2 (close rows 1-3), reconstruct, softmax, einsum2 ----
    def phase2(ci):
        for k in range(KPE, KD):
            nc.tensor.matmul(
                e_ps[ci][:, ci * P:],
                lhsT=vbT[:, k, ci * P:(ci + 1) * P],
                rhs=vbT[:, k, ci * P:],
                start=False,
                stop=(k == KD - 1),
            )

    def prologue(ci):
        """Lower-triangle reconstruct + row softmax through dsc. Emitted
        ahead of the drain so DVE/ACT always have queued work."""
        for cj in range(ci):
            tr_sb = trpool.tile([P, P], F32, name="tr_sb", tag="tr")
            if (ci + cj) % 2 == 0:
                nc.scalar.copy(out=tr_sb[:, :],
                               in_=e_ps[cj][:, ci * P:(ci + 1) * P])
            else:
                nc.vector.tensor_copy(out=tr_sb[:, :],
                                      in_=e_ps[cj][:, ci * P:(ci + 1) * P])
            nc.tensor.matmul(
                e_ps[ci][:, cj * P:(cj + 1) * P],
                lhsT=tr_sb[:, :],
                rhs=identf[:, :],
                is_transpose=True,
                skip_group_check=True,
            )
        nc.vector.tensor_reduce(
            out=mn[:, ci:ci + 1], in_=e_ps[ci][:, :],
            axis=mybir.AxisListType.X, op=mybir.AluOpType.min,
        )
        nc.scalar.activation(
            out=p_sb[:, ci, :], in_=e_ps[ci][:, :],
            func=mybir.ActivationFunctionType.Exp,
            bias=mn[:, ci:ci + 1], scale=-1.0,
            accum_out=zsum[:, ci:ci + 1],
        )
        nc.vector.reciprocal(out=msc[:, ci:ci + 1], in_=zsum[:, ci:ci + 1])
        dsc = dscpool.tile([P, P], BF16, name="dsc", tag="dsc")
        nc.vector.tensor_scalar(
            out=dsc[:, :], in0=identg[:, :], scalar1=msc[:, ci:ci + 1],
            scalar2=None, op0=mybir.AluOpType.mult,
        )
        return dsc

    def emit_pt(ci, dsc):
        # PT block = P_block^T @ diag(gamma/Z) (regular matmul; the
        # transpose datapath ignores rhs values so the scale must go
        # through the normal path)
        pt_ps = opool.tile([P, CI, P], F32, name="pt_ps", tag="op")
        for dj in range(CI):
            nc.tensor.matmul(
                pt_ps[:, dj, :],
                lhsT=p_sb[:, ci, dj * P:(dj + 1) * P],
                rhs=dsc[:, :],
            )
        if ci % 2 == 0:
            nc.scalar.copy(out=pt_sb[:, :, ci * P:(ci + 1) * P],
                           in_=pt_ps[:, :, :])
        else:
            nc.vector.tensor_copy(out=pt_sb[:, :, ci * P:(ci + 1) * P],
                                  in_=pt_ps[:, :, :])

    def emit_groups(ci, gs):
        for g in gs:
            lo = g * OW
            route = ROUTES[ci][g]
            o_ps = opool.tile([P, OW], F32, name="o_ps", tag="op")
            for hb in range(OW // 512):
                hlo = lo + hb * 512
                hsl = slice(hlo, hlo + 512)
                psl = slice(hb * 512, hb * 512 + 512)
                for h in range(CI // 2):
                    nc.tensor.matmul(
                        o_ps[:, psl],
                        lhsT=pt_sb[:, 2 * h:2 * h + 2, ci * P:(ci + 1) * P],
                        rhs=vb8[:, 2 * h:2 * h + 2, hsl],
                        start=(h == 0),
                        stop=(h == CI // 2 - 1 and route == "D"),
                        perf_mode=mybir.MatmulPerfMode.DoubleRow,
                    )
                if route == "A":
                    # fold +x on the PE so the drain is a bare ACT copy
                    nc.tensor.matmul(
                        o_ps[:, psl],
                        lhsT=ident[:, :],
                        rhs=vb[:, ci, hsl],
                        start=False,
                        stop=True,
                    )
            gsl = slice(lo, lo + OW)
            if route == "D":
                nc.vector.tensor_add(out=vb[:, ci, gsl], in0=o_ps[:, :],
                                     in1=xf[:, ci, gsl])
            else:
                nc.scalar.copy(out=vb[:, ci, gsl], in_=o_ps[:, :])
            nc.sync.dma_start(out=out3[:, ci, gsl], in_=vb[:, ci, gsl])

    # Interleaved drain. Emission order IS the static per-engine program
    # order, so: row 0's softmax chain is emitted before phase2 (else the
    # Tile scheduler interleaves rows 1-3 einsum1 into the critical tail),
    # and each row's PT staging tile is emitted right after the previous
    # row's first two groups so the 2-slot PSUM rotation never stalls the
    # row hand-off.
    dsc0 = prologue(0)
    phase2(1)
    dsc1 = prologue(1)
    emit_pt(0, dsc0)
    emit_groups(0, (0, 1))
    phase2(2)
    dsc2 = prologue(2)
    emit_pt(1, dsc1)
    emit_groups(0, (2, 3))
    emit_groups(1, (0, 1))
    phase2(3)
    dsc3 = prologue(3)
    emit_pt(2, dsc2)
    emit_groups(1, (2, 3))
    emit_groups(2, (0, 1))
    emit_pt(3, dsc3)
    emit_groups(2, (2, 3))
    emit_groups(3, (0, 1, 2, 3))


def build():
    nc = bacc.Bacc("TRN2", target_bir_lowering=False, debug=False,
                   num_devices=N_CORES)
    x = nc.dram_tensor("x", [C, S], F32, kind="ExternalInput")
    gamma = nc.dram_tensor("gamma", [1], F32, kind="ExternalInput")
    out = nc.dram_tensor("out", [C, S], BF16, kind="ExternalOutput")
    with tile.TileContext(nc) as tc:
        with ExitStack() as ctx:
            _body(ctx, tc, out.ap(), x.ap(), gamma.ap())
    nc.compile()
    return nc


_NC_CACHE = {}
LAST_RESULTS = None


def kernel(x: np.ndarray, gamma: np.ndarray) -> np.ndarray:
    global LAST_RESULTS
    x = np.ascontiguousarray(np.asarray(x, dtype=np.float32))
    gamma = np.ascontiguousarray(np.asarray(gamma, dtype=np.float32))
    n, c, h, w = x.shape
    assert (n, c, h * w) == (N_CORES, C, S), f"unexpected shape {x.shape}"

    os.environ["BASS_NEVER_TRACE"] = "1"

    if "nc" not in _NC_CACHE:
        _NC_CACHE["nc"] = build()
    nc = _NC_CACHE["nc"]

    in_maps = [
        {"x": x[i].reshape(C, S), "gamma": gamma} for i in range(N_CORES)
    ]
    res = run_bass_kernel_spmd(nc, in_maps, core_ids=list(range(N_CORES)))
    LAST_RESULTS = res
    out = np.stack(
        [np.asarray(res.results[i]["out"]) for i in range(N_CORES)], axis=0
    ).astype(np.float32)
    return out.reshape(n, c, h, w)


if __name__ == "__main__":
    xs = np.random.randn(N_CORES, C, 64, 64).astype(np.float32)
    g = np.zeros((1,), np.float32)
    o = kernel(xs, g)
    print("ok", o.shape, np.abs(o - xs).max())


# revision 16
# speedup vs baseline: 1.0154x; 1.0154x over previous
"""Channel-attention (CAM) Trainium2 Bass kernel.

Reference computation (per batch n):
    v = x[n].reshape(C, S)                 # C=512, S=H*W=4096
    energy = v @ v.T                       # (C, C)
    att = softmax(max_row(energy) - energy, axis=-1)
        = exp(min_row(energy) - energy) / Z
    out[n] = gamma * (att @ v) + x[n]

Sharding: data-parallel over N=8 batches across 8 NeuronCores; each core
computes one full C x C attention locally (no collectives).

v3 design (baseline 61377ns -> target ~41us). Two structural changes:

1. BF16 output (rel-err ~2e-3 vs the 2e-2 gate) halves store DMA
   23.3us -> 11.65us, which makes the drain ENGINE-bound: every einsum2
   group must cross PSUM->SBUF on DVE or ACT (gpsimd has no PSUM port,
   DMA cannot read PSUM, PE cannot read PSUM).

2. einsum2 groups accumulate in BF16 PSUM, 1024 wide. Consequences:
   - a group is ONE PSUM bank, so the shared staging tag rotates 4 deep
     (vs 2 with f32 groups) and the fill->drain->refill cycle pipelines;
   - DVE epilogues hit the 2x_1p fast mode (all operands 2-byte):
     ~818ns per 1024 instead of 1352;
   - precision: with gamma=0 the group holds exactly bf16(x) (identity
     matmul of bf16 x, no rounding loss); nonzero gamma adds bf16
     rounding of the attention part, well inside the 2e-1 regime.

   Route 'D' groups: one DVE tensor_add (bf16 PSUM + vb -> vb, 2x mode).
   Route 'A' groups: PE identity-matmul folds +x into PSUM (~245ns),
   then one bare ACT copy PSUM -> vb.

Other structure:
   - the energy softmax prologues (row-min DVE, exp ACT with Z-accum,
     dsc = gamma*I/Z) are emitted two rows AHEAD of the einsum2 drain so
     both engines always have independent queued work;
   - all einsum1 tail chunks run row-0-only inline during the loads
     (KPE chunks run all 4 rows), so row 0 closes right after the last
     load; rows 1-3 finish in phase 2 (PE, overlapped with softmax 0);
   - PSUM->vbT transpose staging moves in PAIRS of chunks (one
     1024-wide copy per two chunks) alternating DVE/ACT;
   - gpsimd carries the fp8 casts of v (einsum2 rhs), a few on ACT;
   - PSUM: 4 banks energy + 4 rotating 2KB slots shared by transpose
     pairs, bf16 einsum2 groups and the PT staging tile.
"""

import os
from contextlib import ExitStack

import numpy as np

import concourse.bass as bass
import concourse.tile as tile
from concourse import bacc, mybir
from concourse.bass_utils import run_bass_kernel_spmd
from concourse.masks import make_identity

N_CORES = 8
C = 512
S = 4096
P = 128
CI = C // P  # 4 c-chunks
KD = S // P  # 32 s-chunks of 128
OW = 1024    # einsum2 output group width (1 bf16 PSUM bank)
OG = S // OW  # 4 groups per row

STRIPES = [512] * 7 + [384, 128]
KPE = 16        # chunks whose einsum1 runs all 4 rows inline during loads
TAILF32 = 8     # trailing chunks transposed from xf in f32 (no vb cast dep)
SINGLE_ENG = list("AAAADADA")  # copy engine per tail chunk (KD-TAILF32..KD-1)
# einsum2 route per (row, group): 'D' = DVE add (PSUM+xf->vb), 'A' = PE
# identity-add + ACT copy. g3 must be 'D': its columns overlap the TAILF32
# region, which never gets a bf16 cast into vb.
ROUTES = (
    ("D", "A", "A", "D"),
    ("A", "D", "A", "D"),
    ("A", "D", "A", "D"),
    ("A", "A", "D", "D"),
)
# engine for each vbT pair copy: 'D' (DVE) / 'A' (ACT)
PAIR_ENG = list("DADADADADADADADA")
# engine for each of the 36 (stripe, ci) fp8 casts: gpsimd with some ACT
VB8_ENG = list("GGGGGGGGGGGGGGGGAGGGAGGGAGGGAGGGAGGG")

F32 = mybir.dt.float32
BF16 = mybir.dt.bfloat16
FP8 = mybir.dt.float8e4


def _body(ctx: ExitStack, tc: tile.TileContext, out: bass.AP, x: bass.AP,
          gamma: bass.AP):
    nc = tc.nc

    persist = ctx.enter_context(tc.tile_pool(name="persist", bufs=1))
    xf = persist.tile([P, CI, S], F32, name="xf")
    vb = persist.tile([P, CI, S], BF16, name="vb")   # bf16 x, then output
    vb8 = persist.tile([P, CI, S], FP8, name="vb8")
    vbT = persist.tile([P, KD, C], BF16, name="vbT")
    p_sb = persist.tile([P, CI, C], BF16, name="p_sb")
    pt_sb = persist.tile([P, CI, C], FP8, name="pt_sb")
    ident = persist.tile([P, P], BF16, name="ident")
    identf = persist.tile([P, P], F32, name="identf")
    identg = persist.tile([P, P], BF16, name="identg")
    gamma_sb = persist.tile([P, 1], F32, name="gamma_sb")
    mn = persist.tile([P, CI], F32, name="mn")
    zsum = persist.tile([P, CI], F32, name="zsum")
    msc = persist.tile([P, CI], F32, name="msc")

    make_identity(nc, ident)
    make_identity(nc, identf)

    x3 = x.rearrange("(ci p) s -> p ci s", p=P)
    out3 = out.rearrange("(ci p) s -> p ci s", p=P)

    epool = ctx.enter_context(tc.tile_pool(name="epool", bufs=4, space="PSUM"))
    e_ps = [epool.tile([P, C], F32, name=f"e{ci}", tag="et") for ci in range(CI)]
    # 2 rotating 4KB slots shared (by tag) between transpose staging pairs,
    # f32 einsum2 groups and the PT staging tile.
    opool = ctx.enter_context(tc.tile_pool(name="opool", bufs=2, space="PSUM"))
    trpool = ctx.enter_context(tc.tile_pool(name="trp", bufs=3))
    dscpool = ctx.enter_context(tc.tile_pool(name="dscp", bufs=2))

    # ---- load + cast + PE-transpose + inline einsum1 ----
    def emit_e1(k):
        rows = range(CI) if k < KPE else (0,)
        for ci in rows:
            nc.tensor.matmul(
                e_ps[ci][:, ci * P:],
                lhsT=vbT[:, k, ci * P:(ci + 1) * P],
                rhs=vbT[:, k, ci * P:],
                start=(k == 0),
                stop=(k == KD - 1),
            )

    tp_cur = None
    col = 0
    for si, w in enumerate(STRIPES):
        sl = slice(col, col + w)
        for ci in range(CI):
            nc.sync.dma_start(out=xf[:, ci, sl], in_=x3[:, ci, sl])
            cast_hi = min(col + w, (KD - TAILF32) * P)
            if col < cast_hi:
                with tc.high_priority():
                    nc.vector.tensor_copy(out=vb[:, ci, col:cast_hi],
                                          in_=xf[:, ci, col:cast_hi])
            eng = VB8_ENG[si * CI + ci]
            if eng == "A":
                nc.scalar.copy(out=vb8[:, ci, sl], in_=xf[:, ci, sl])
            else:
                nc.gpsimd.tensor_copy(out=vb8[:, ci, sl], in_=xf[:, ci, sl])
        if si == 0:
            nc.sync.dma_start(out=gamma_sb[:, :],
                              in_=gamma.to_broadcast((P, 1)))
            # gamma*I once; dsc = identg * (1/Z) per row
            nc.vector.tensor_scalar(
                out=identg[:, :], in0=ident[:, :], scalar1=gamma_sb[:, :],
                scalar2=None, op0=mybir.AluOpType.mult,
            )
        for k in range(col // P, (col + w) // P):
            tailf = k >= KD - TAILF32
            j, half = k // 2, k % 2
            if tailf:
                # single-chunk staging: 2KB tiles ping-pong through the
                # 2-slot rotation at ~700ns/chunk instead of ~1.9us/pair
                tp_cur = opool.tile([P, C], F32, name="tps", tag="op")
                for ci in range(CI):
                    nc.tensor.transpose(
                        out=tp_cur[:, ci * P:(ci + 1) * P],
                        in_=xf[:, ci, k * P:(k + 1) * P],
                        identity=identf[:, :],
                    )
                if SINGLE_ENG[k - (KD - TAILF32)] == "A":
                    nc.scalar.copy(out=vbT[:, k, :], in_=tp_cur[:, :])
                else:
                    nc.vector.tensor_copy(out=vbT[:, k, :], in_=tp_cur[:, :])
                emit_e1(k - 2)
                continue
            if half == 0:
                tp_cur = opool.tile([P, 2, C], F32 if tailf else BF16,
                                    name="tp", tag="op")
            for ci in range(CI):
                nc.tensor.transpose(
                    out=tp_cur[:, half, ci * P:(ci + 1) * P],
                    in_=(xf if tailf else vb)[:, ci, k * P:(k + 1) * P],
                    identity=(identf if tailf else ident)[:, :],
                )
            if half == 1:
                # one 1024-wide PSUM->SBUF copy moves the whole pair
                if PAIR_ENG[j] == "D":
                    nc.vector.tensor_copy(out=vbT[:, 2 * j:2 * j + 2, :],
                                          in_=tp_cur[:, :, :])
                else:
                    nc.scalar.copy(out=vbT[:, 2 * j:2 * j + 2, :],
                                   in_=tp_cur[:, :, :])
                # einsum1 for the PREVIOUS pair (its copy landed during
                # this pair's transposes -- the in-order PE never stalls
                # on a copy it just triggered)
                if j > 0:
                    emit_e1(2 * (j - 1))
                    emit_e1(2 * (j - 1) + 1)
        col += w
    emit_e1(KD - 2)
    emit_e1(KD - 1)

    # ---- phase 2 (close rows 1-3), reconstruct, softmax, einsum2 ----
    def phase2(ci):
        for k in range(KPE, KD):
            nc.tensor.matmul(
                e_ps[ci][:, ci * P:],
                lhsT=vbT[:, k, ci * P:(ci + 1) * P],
                rhs=vbT[:, k, ci * P:],
                start=False,
                stop=(k == KD - 1),
            )

    def emit_tr(ci):
        """Lower-triangle reconstruct for row ci (all its upper sources
        are closed). Emitted as early as possible so the source banks die
        early enough for PT staging to reuse them."""
        for cj in range(ci):
            tr_sb = trpool.tile([P, P], F32, name="tr_sb", tag="tr")
            if (ci + cj) % 2 == 0:
                nc.scalar.copy(out=tr_sb[:, :],
                               in_=e_ps[cj][:, ci * P:(ci + 1) * P])
            else:
                nc.vector.tensor_copy(out=tr_sb[:, :],
                                      in_=e_ps[cj][:, ci * P:(ci + 1) * P])
            nc.tensor.matmul(
                e_ps[ci][:, cj * P:(cj + 1) * P],
                lhsT=tr_sb[:, :],
                rhs=identf[:, :],
                is_transpose=True,
                skip_group_check=True,
            )

    def prologue(ci):
        """Row softmax through dsc. Emitted ahead of the drain so
        DVE/ACT always have queued work."""
        nc.vector.tensor_reduce(
            out=mn[:, ci:ci + 1], in_=e_ps[ci][:, :],
            axis=mybir.AxisListType.X, op=mybir.AluOpType.min,
        )
        nc.scalar.activation(
            out=p_sb[:, ci, :], in_=e_ps[ci][:, :],
            func=mybir.ActivationFunctionType.Exp,
            bias=mn[:, ci:ci + 1], scale=-1.0,
            accum_out=zsum[:, ci:ci + 1],
        )
        nc.vector.reciprocal(out=msc[:, ci:ci + 1], in_=zsum[:, ci:ci + 1])
        dsc = dscpool.tile([P, P], BF16, name="dsc", tag="dsc")
        nc.vector.tensor_scalar(
            out=dsc[:, :], in0=identg[:, :], scalar1=msc[:, ci:ci + 1],
            scalar2=None, op0=mybir.AluOpType.mult,
        )
        return dsc

    def emit_pt(ci, dsc, bank=None):
        # PT block = P_block^T @ diag(gamma/Z) (regular matmul; the
        # transpose datapath ignores rhs values so the scale must go
        # through the normal path). PT for rows 1-3 writes into a DEAD
        # energy bank (all its readers ran), keeping the op-slot FIFO
        # free for einsum2 groups; only row 0's PT takes a FIFO turn.
        if bank is None:
            pt_ps = opool.tile([P, CI, P], F32, name="pt_ps", tag="op")
        else:
            pt_ps = bank.rearrange("p (dj q) -> p dj q", dj=CI)
        for dj in range(CI):
            nc.tensor.matmul(
                pt_ps[:, dj, :],
                lhsT=p_sb[:, ci, dj * P:(dj + 1) * P],
                rhs=dsc[:, :],
                skip_group_check=bank is not None,
            )
        if ci % 2 == 0:
            nc.scalar.copy(out=pt_sb[:, :, ci * P:(ci + 1) * P],
                           in_=pt_ps[:, :, :])
        else:
            nc.vector.tensor_copy(out=pt_sb[:, :, ci * P:(ci + 1) * P],
                                  in_=pt_ps[:, :, :])

    def emit_groups(ci, gs):
        for g in gs:
            lo = g * OW
            route = ROUTES[ci][g]
            o_ps = opool.tile([P, OW], F32, name="o_ps", tag="op")
            for hb in range(OW // 512):
                hlo = lo + hb * 512
                hsl = slice(hlo, hlo + 512)
                psl = slice(hb * 512, hb * 512 + 512)
                for h in range(CI // 2):
                    nc.tensor.matmul(
                        o_ps[:, psl],
                        lhsT=pt_sb[:, 2 * h:2 * h + 2, ci * P:(ci + 1) * P],
                        rhs=vb8[:, 2 * h:2 * h + 2, hsl],
                        start=(h == 0),
                        stop=(h == CI // 2 - 1 and route == "D"),
                        perf_mode=mybir.MatmulPerfMode.DoubleRow,
                    )
                if route == "A":
                    # fold +x on the PE so the drain is a bare ACT copy
                    nc.tensor.matmul(
                        o_ps[:, psl],
                        lhsT=ident[:, :],
                        rhs=vb[:, ci, hsl],
                        start=False,
                        stop=True,
                    )
            gsl = slice(lo, lo + OW)
            if route == "D":
                nc.vector.tensor_add(out=vb[:, ci, gsl], in0=o_ps[:, :],
                                     in1=xf[:, ci, gsl])
            else:
                nc.scalar.copy(out=vb[:, ci, gsl], in_=o_ps[:, :])
            nc.sync.dma_start(out=out3[:, ci, gsl], in_=vb[:, ci, gsl])

    # Interleaved drain. Emission order IS the static per-engine program
    # order, so: row 0's softmax chain is emitted before phase2 (else the
    # Tile scheduler interleaves rows 1-3 einsum1 into the critical tail),
    # and each row's PT staging tile is emitted right after the previous
    # row's first two groups so the 2-slot PSUM rotation never stalls the
    # row hand-off.
    dsc0 = prologue(0)
    phase2(1)
    emit_tr(1)
    dsc1 = prologue(1)
    emit_pt(0, dsc0)
    emit_groups(0, (0, 1))
    phase2(2)
    emit_tr(2)
    dsc2 = prologue(2)
    phase2(3)
    emit_tr(3)
    # e0's readers (exp0 + all tr reads) are done: PT1 reuses its bank
    emit_pt(1, dsc1, bank=e_ps[0])
    emit_groups(0, (2, 3))
    emit_groups(1, (0, 1))
    dsc3 = prologue(3)
    emit_pt(2, dsc2, bank=e_ps[1])
    emit_groups(1, (2, 3))
    emit_groups(2, (0, 1))
    emit_pt(3, dsc3, bank=e_ps[2])
    emit_groups(2, (2, 3))
    emit_groups(3, (0, 1, 2, 3))


def build():
    nc = bacc.Bacc("TRN2", target_bir_lowering=False, debug=False,
                   num_devices=N_CORES)
    x = nc.dram_tensor("x", [C, S], F32, kind="ExternalInput")
    gamma = nc.dram_tensor("gamma", [1], F32, kind="ExternalInput")
    out = nc.dram_tensor("out", [C, S], BF16, kind="ExternalOutput")
    with tile.TileContext(nc) as tc:
        with ExitStack() as ctx:
            _body(ctx, tc, out.ap(), x.ap(), gamma.ap())
    nc.compile()
    return nc


_NC_CACHE = {}
LAST_RESULTS = None


def kernel(x: np.ndarray, gamma: np.ndarray) -> np.ndarray:
    global LAST_RESULTS
    x = np.ascontiguousarray(np.asarray(x, dtype=np.float32))
    gamma = np.ascontiguousarray(np.asarray(gamma, dtype=np.float32))
    n, c, h, w = x.shape
    assert (n, c, h * w) == (N_CORES, C, S), f"unexpected shape {x.shape}"

    os.environ["BASS_NEVER_TRACE"] = "1"

    if "nc" not in _NC_CACHE:
        _NC_CACHE["nc"] = build()
    nc = _NC_CACHE["nc"]

    in_maps = [
        {"x": x[i].reshape(C, S), "gamma": gamma} for i in range(N_CORES)
    ]
    res = run_bass_kernel_spmd(nc, in_maps, core_ids=list(range(N_CORES)))
    LAST_RESULTS = res
    out = np.stack(
        [np.asarray(res.results[i]["out"]) for i in range(N_CORES)], axis=0
    ).astype(np.float32)
    return out.reshape(n, c, h, w)


if __name__ == "__main__":
    xs = np.random.randn(N_CORES, C, 64, 64).astype(np.float32)
    g = np.zeros((1,), np.float32)
    o = kernel(xs, g)
    print("ok", o.shape, np.abs(o - xs).max())


# revision 17
# speedup vs baseline: 1.0191x; 1.0036x over previous
"""Channel-attention (CAM) Trainium2 Bass kernel.

Reference computation (per batch n):
    v = x[n].reshape(C, S)                 # C=512, S=H*W=4096
    energy = v @ v.T                       # (C, C)
    att = softmax(max_row(energy) - energy, axis=-1)
        = exp(min_row(energy) - energy) / Z
    out[n] = gamma * (att @ v) + x[n]

Sharding: data-parallel over N=8 batches across 8 NeuronCores; each core
computes one full C x C attention locally (no collectives).

v3 design (baseline 61377ns -> target ~41us). Two structural changes:

1. BF16 output (rel-err ~2e-3 vs the 2e-2 gate) halves store DMA
   23.3us -> 11.65us, which makes the drain ENGINE-bound: every einsum2
   group must cross PSUM->SBUF on DVE or ACT (gpsimd has no PSUM port,
   DMA cannot read PSUM, PE cannot read PSUM).

2. einsum2 groups accumulate in BF16 PSUM, 1024 wide. Consequences:
   - a group is ONE PSUM bank, so the shared staging tag rotates 4 deep
     (vs 2 with f32 groups) and the fill->drain->refill cycle pipelines;
   - DVE epilogues hit the 2x_1p fast mode (all operands 2-byte):
     ~818ns per 1024 instead of 1352;
   - precision: with gamma=0 the group holds exactly bf16(x) (identity
     matmul of bf16 x, no rounding loss); nonzero gamma adds bf16
     rounding of the attention part, well inside the 2e-1 regime.

   Route 'D' groups: one DVE tensor_add (bf16 PSUM + vb -> vb, 2x mode).
   Route 'A' groups: PE identity-matmul folds +x into PSUM (~245ns),
   then one bare ACT copy PSUM -> vb.

Other structure:
   - the energy softmax prologues (row-min DVE, exp ACT with Z-accum,
     dsc = gamma*I/Z) are emitted two rows AHEAD of the einsum2 drain so
     both engines always have independent queued work;
   - all einsum1 tail chunks run row-0-only inline during the loads
     (KPE chunks run all 4 rows), so row 0 closes right after the last
     load; rows 1-3 finish in phase 2 (PE, overlapped with softmax 0);
   - PSUM->vbT transpose staging moves in PAIRS of chunks (one
     1024-wide copy per two chunks) alternating DVE/ACT;
   - gpsimd carries the fp8 casts of v (einsum2 rhs), a few on ACT;
   - PSUM: 4 banks energy + 4 rotating 2KB slots shared by transpose
     pairs, bf16 einsum2 groups and the PT staging tile.
"""

import os
from contextlib import ExitStack

import numpy as np

import concourse.bass as bass
import concourse.tile as tile
from concourse import bacc, mybir
from concourse.bass_utils import run_bass_kernel_spmd
from concourse.masks import make_identity

N_CORES = 8
C = 512
S = 4096
P = 128
CI = C // P  # 4 c-chunks
KD = S // P  # 32 s-chunks of 128
OW = 1024    # einsum2 output group width (1 bf16 PSUM bank)
OG = S // OW  # 4 groups per row

STRIPES = [512] * 7 + [384, 128]
KPE = 18        # chunks whose einsum1 runs all 4 rows inline during loads
TAILF32 = 8     # trailing chunks transposed from xf in f32 (no vb cast dep)
SINGLE_ENG = list("AAAADADA")  # copy engine per tail chunk (KD-TAILF32..KD-1)
# einsum2 route per (row, group): 'D' = DVE add (PSUM+xf->vb), 'A' = PE
# identity-add + ACT copy. g3 must be 'D': its columns overlap the TAILF32
# region, which never gets a bf16 cast into vb.
ROUTES = (
    ("D", "A", "A", "D"),
    ("A", "D", "A", "D"),
    ("A", "D", "A", "D"),
    ("A", "A", "D", "D"),
)
# engine for each vbT pair copy: 'D' (DVE) / 'A' (ACT)
PAIR_ENG = list("DADADADADADADADA")
# engine for each of the 36 (stripe, ci) fp8 casts: gpsimd with some ACT
VB8_ENG = list("GGGGGGGGGGGGGGGGAGGGAGGGAGGGAGGGAGGG")

F32 = mybir.dt.float32
BF16 = mybir.dt.bfloat16
FP8 = mybir.dt.float8e4


def _body(ctx: ExitStack, tc: tile.TileContext, out: bass.AP, x: bass.AP,
          gamma: bass.AP):
    nc = tc.nc

    persist = ctx.enter_context(tc.tile_pool(name="persist", bufs=1))
    xf = persist.tile([P, CI, S], F32, name="xf")
    vb = persist.tile([P, CI, S], BF16, name="vb")   # bf16 x, then output
    vb8 = persist.tile([P, CI, S], FP8, name="vb8")
    vbT = persist.tile([P, KD, C], BF16, name="vbT")
    p_sb = persist.tile([P, CI, C], BF16, name="p_sb")
    pt_sb = persist.tile([P, CI, C], FP8, name="pt_sb")
    ident = persist.tile([P, P], BF16, name="ident")
    identf = persist.tile([P, P], F32, name="identf")
    identg = persist.tile([P, P], BF16, name="identg")
    gamma_sb = persist.tile([P, 1], F32, name="gamma_sb")
    mn = persist.tile([P, CI], F32, name="mn")
    zsum = persist.tile([P, CI], F32, name="zsum")
    msc = persist.tile([P, CI], F32, name="msc")

    make_identity(nc, ident)
    make_identity(nc, identf)

    x3 = x.rearrange("(ci p) s -> p ci s", p=P)
    out3 = out.rearrange("(ci p) s -> p ci s", p=P)

    epool = ctx.enter_context(tc.tile_pool(name="epool", bufs=4, space="PSUM"))
    e_ps = [epool.tile([P, C], F32, name=f"e{ci}", tag="et") for ci in range(CI)]
    # 2 rotating 4KB slots shared (by tag) between transpose staging pairs,
    # f32 einsum2 groups and the PT staging tile.
    opool = ctx.enter_context(tc.tile_pool(name="opool", bufs=2, space="PSUM"))
    trpool = ctx.enter_context(tc.tile_pool(name="trp", bufs=3))
    dscpool = ctx.enter_context(tc.tile_pool(name="dscp", bufs=2))

    # ---- load + cast + PE-transpose + inline einsum1 ----
    def emit_e1(k):
        rows = range(CI) if k < KPE else (0,)
        for ci in rows:
            nc.tensor.matmul(
                e_ps[ci][:, ci * P:],
                lhsT=vbT[:, k, ci * P:(ci + 1) * P],
                rhs=vbT[:, k, ci * P:],
                start=(k == 0),
                stop=(k == KD - 1),
            )

    tp_cur = None
    col = 0
    for si, w in enumerate(STRIPES):
        sl = slice(col, col + w)
        for ci in range(CI):
            nc.sync.dma_start(out=xf[:, ci, sl], in_=x3[:, ci, sl])
            cast_hi = min(col + w, (KD - TAILF32) * P)
            if col < cast_hi:
                with tc.high_priority():
                    nc.vector.tensor_copy(out=vb[:, ci, col:cast_hi],
                                          in_=xf[:, ci, col:cast_hi])
            eng = VB8_ENG[si * CI + ci]
            if eng == "A":
                nc.scalar.copy(out=vb8[:, ci, sl], in_=xf[:, ci, sl])
            else:
                nc.gpsimd.tensor_copy(out=vb8[:, ci, sl], in_=xf[:, ci, sl])
        if si == 0:
            nc.sync.dma_start(out=gamma_sb[:, :],
                              in_=gamma.to_broadcast((P, 1)))
            # gamma*I once; dsc = identg * (1/Z) per row
            nc.vector.tensor_scalar(
                out=identg[:, :], in0=ident[:, :], scalar1=gamma_sb[:, :],
                scalar2=None, op0=mybir.AluOpType.mult,
            )
        for k in range(col // P, (col + w) // P):
            tailf = k >= KD - TAILF32
            j, half = k // 2, k % 2
            if tailf:
                # single-chunk staging: 2KB tiles ping-pong through the
                # 2-slot rotation at ~700ns/chunk instead of ~1.9us/pair
                tp_cur = opool.tile([P, C], F32, name="tps", tag="op")
                for ci in range(CI):
                    nc.tensor.transpose(
                        out=tp_cur[:, ci * P:(ci + 1) * P],
                        in_=xf[:, ci, k * P:(k + 1) * P],
                        identity=identf[:, :],
                    )
                if SINGLE_ENG[k - (KD - TAILF32)] == "A":
                    nc.scalar.copy(out=vbT[:, k, :], in_=tp_cur[:, :])
                else:
                    nc.vector.tensor_copy(out=vbT[:, k, :], in_=tp_cur[:, :])
                emit_e1(k - 2)
                continue
            if half == 0:
                tp_cur = opool.tile([P, 2, C], F32 if tailf else BF16,
                                    name="tp", tag="op")
            for ci in range(CI):
                nc.tensor.transpose(
                    out=tp_cur[:, half, ci * P:(ci + 1) * P],
                    in_=(xf if tailf else vb)[:, ci, k * P:(k + 1) * P],
                    identity=(identf if tailf else ident)[:, :],
                )
            if half == 1:
                # one 1024-wide PSUM->SBUF copy moves the whole pair
                if PAIR_ENG[j] == "D":
                    nc.vector.tensor_copy(out=vbT[:, 2 * j:2 * j + 2, :],
                                          in_=tp_cur[:, :, :])
                else:
                    nc.scalar.copy(out=vbT[:, 2 * j:2 * j + 2, :],
                                   in_=tp_cur[:, :, :])
                # einsum1 for the PREVIOUS pair (its copy landed during
                # this pair's transposes -- the in-order PE never stalls
                # on a copy it just triggered)
                if j > 0:
                    emit_e1(2 * (j - 1))
                    emit_e1(2 * (j - 1) + 1)
        col += w
    emit_e1(KD - 2)
    emit_e1(KD - 1)

    # ---- phase 2 (close rows 1-3), reconstruct, softmax, einsum2 ----
    def phase2(ci):
        for k in range(KPE, KD):
            nc.tensor.matmul(
                e_ps[ci][:, ci * P:],
                lhsT=vbT[:, k, ci * P:(ci + 1) * P],
                rhs=vbT[:, k, ci * P:],
                start=False,
                stop=(k == KD - 1),
            )

    def emit_tr(ci):
        """Lower-triangle reconstruct for row ci (all its upper sources
        are closed). Emitted as early as possible so the source banks die
        early enough for PT staging to reuse them."""
        for cj in range(ci):
            tr_sb = trpool.tile([P, P], F32, name="tr_sb", tag="tr")
            if (ci + cj) % 2 == 0:
                nc.scalar.copy(out=tr_sb[:, :],
                               in_=e_ps[cj][:, ci * P:(ci + 1) * P])
            else:
                nc.vector.tensor_copy(out=tr_sb[:, :],
                                      in_=e_ps[cj][:, ci * P:(ci + 1) * P])
            nc.tensor.matmul(
                e_ps[ci][:, cj * P:(cj + 1) * P],
                lhsT=tr_sb[:, :],
                rhs=identf[:, :],
                is_transpose=True,
                skip_group_check=True,
            )

    def prologue(ci):
        """Row softmax through dsc. Emitted ahead of the drain so
        DVE/ACT always have queued work."""
        nc.vector.tensor_reduce(
            out=mn[:, ci:ci + 1], in_=e_ps[ci][:, :],
            axis=mybir.AxisListType.X, op=mybir.AluOpType.min,
        )
        nc.scalar.activation(
            out=p_sb[:, ci, :], in_=e_ps[ci][:, :],
            func=mybir.ActivationFunctionType.Exp,
            bias=mn[:, ci:ci + 1], scale=-1.0,
            accum_out=zsum[:, ci:ci + 1],
        )
        nc.vector.reciprocal(out=msc[:, ci:ci + 1], in_=zsum[:, ci:ci + 1])
        dsc = dscpool.tile([P, P], BF16, name="dsc", tag="dsc")
        nc.vector.tensor_scalar(
            out=dsc[:, :], in0=identg[:, :], scalar1=msc[:, ci:ci + 1],
            scalar2=None, op0=mybir.AluOpType.mult,
        )
        return dsc

    def emit_pt(ci, dsc, bank=None):
        # PT block = P_block^T @ diag(gamma/Z) (regular matmul; the
        # transpose datapath ignores rhs values so the scale must go
        # through the normal path). PT for rows 1-3 writes into a DEAD
        # energy bank (all its readers ran), keeping the op-slot FIFO
        # free for einsum2 groups; only row 0's PT takes a FIFO turn.
        if bank is None:
            pt_ps = opool.tile([P, CI, P], F32, name="pt_ps", tag="op")
        else:
            pt_ps = bank.rearrange("p (dj q) -> p dj q", dj=CI)
        for dj in range(CI):
            nc.tensor.matmul(
                pt_ps[:, dj, :],
                lhsT=p_sb[:, ci, dj * P:(dj + 1) * P],
                rhs=dsc[:, :],
                skip_group_check=bank is not None,
            )
        if ci % 2 == 0:
            nc.scalar.copy(out=pt_sb[:, :, ci * P:(ci + 1) * P],
                           in_=pt_ps[:, :, :])
        else:
            nc.vector.tensor_copy(out=pt_sb[:, :, ci * P:(ci + 1) * P],
                                  in_=pt_ps[:, :, :])

    def emit_groups(ci, gs):
        for g in gs:
            lo = g * OW
            route = ROUTES[ci][g]
            o_ps = opool.tile([P, OW], F32, name="o_ps", tag="op")
            for hb in range(OW // 512):
                hlo = lo + hb * 512
                hsl = slice(hlo, hlo + 512)
                psl = slice(hb * 512, hb * 512 + 512)
                for h in range(CI // 2):
                    nc.tensor.matmul(
                        o_ps[:, psl],
                        lhsT=pt_sb[:, 2 * h:2 * h + 2, ci * P:(ci + 1) * P],
                        rhs=vb8[:, 2 * h:2 * h + 2, hsl],
                        start=(h == 0),
                        stop=(h == CI // 2 - 1 and route == "D"),
                        perf_mode=mybir.MatmulPerfMode.DoubleRow,
                    )
                if route == "A":
                    # fold +x on the PE so the drain is a bare ACT copy
                    nc.tensor.matmul(
                        o_ps[:, psl],
                        lhsT=ident[:, :],
                        rhs=vb[:, ci, hsl],
                        start=False,
                        stop=True,
                    )
            gsl = slice(lo, lo + OW)
            if route == "D":
                nc.vector.tensor_add(out=vb[:, ci, gsl], in0=o_ps[:, :],
                                     in1=xf[:, ci, gsl])
            else:
                nc.scalar.copy(out=vb[:, ci, gsl], in_=o_ps[:, :])
            nc.sync.dma_start(out=out3[:, ci, gsl], in_=vb[:, ci, gsl])

    # Interleaved drain. Emission order IS the static per-engine program
    # order, so: row 0's softmax chain is emitted before phase2 (else the
    # Tile scheduler interleaves rows 1-3 einsum1 into the critical tail),
    # and each row's PT staging tile is emitted right after the previous
    # row's first two groups so the 2-slot PSUM rotation never stalls the
    # row hand-off.
    dsc0 = prologue(0)
    phase2(1)
    emit_tr(1)
    dsc1 = prologue(1)
    emit_pt(0, dsc0)
    emit_groups(0, (0, 1))
    phase2(2)
    emit_tr(2)
    dsc2 = prologue(2)
    phase2(3)
    emit_tr(3)
    # e0's readers (exp0 + all tr reads) are done: PT1 reuses its bank
    emit_pt(1, dsc1, bank=e_ps[0])
    emit_groups(0, (2, 3))
    emit_groups(1, (0, 1))
    dsc3 = prologue(3)
    emit_pt(2, dsc2, bank=e_ps[1])
    emit_groups(1, (2, 3))
    emit_groups(2, (0, 1))
    emit_pt(3, dsc3, bank=e_ps[2])
    emit_groups(2, (2, 3))
    emit_groups(3, (0, 1, 2, 3))


def build():
    nc = bacc.Bacc("TRN2", target_bir_lowering=False, debug=False,
                   num_devices=N_CORES)
    x = nc.dram_tensor("x", [C, S], F32, kind="ExternalInput")
    gamma = nc.dram_tensor("gamma", [1], F32, kind="ExternalInput")
    out = nc.dram_tensor("out", [C, S], BF16, kind="ExternalOutput")
    with tile.TileContext(nc) as tc:
        with ExitStack() as ctx:
            _body(ctx, tc, out.ap(), x.ap(), gamma.ap())
    nc.compile()
    return nc


_NC_CACHE = {}
LAST_RESULTS = None


def kernel(x: np.ndarray, gamma: np.ndarray) -> np.ndarray:
    global LAST_RESULTS
    x = np.ascontiguousarray(np.asarray(x, dtype=np.float32))
    gamma = np.ascontiguousarray(np.asarray(gamma, dtype=np.float32))
    n, c, h, w = x.shape
    assert (n, c, h * w) == (N_CORES, C, S), f"unexpected shape {x.shape}"

    os.environ["BASS_NEVER_TRACE"] = "1"

    if "nc" not in _NC_CACHE:
        _NC_CACHE["nc"] = build()
    nc = _NC_CACHE["nc"]

    in_maps = [
        {"x": x[i].reshape(C, S), "gamma": gamma} for i in range(N_CORES)
    ]
    res = run_bass_kernel_spmd(nc, in_maps, core_ids=list(range(N_CORES)))
    LAST_RESULTS = res
    out = np.stack(
        [np.asarray(res.results[i]["out"]) for i in range(N_CORES)], axis=0
    ).astype(np.float32)
    return out.reshape(n, c, h, w)


if __name__ == "__main__":
    xs = np.random.randn(N_CORES, C, 64, 64).astype(np.float32)
    g = np.zeros((1,), np.float32)
    o = kernel(xs, g)
    print("ok", o.shape, np.abs(o - xs).max())


# revision 18
# speedup vs baseline: 1.0327x; 1.0134x over previous
"""Channel-attention (CAM) Trainium2 Bass kernel.

Reference computation (per batch n):
    v = x[n].reshape(C, S)                 # C=512, S=H*W=4096
    energy = v @ v.T                       # (C, C)
    att = softmax(max_row(energy) - energy, axis=-1)
        = exp(min_row(energy) - energy) / Z
    out[n] = gamma * (att @ v) + x[n]

Sharding: data-parallel over N=8 batches across 8 NeuronCores; each core
computes one full C x C attention locally (no collectives).

v3 design (baseline 61377ns -> target ~41us). Two structural changes:

1. BF16 output (rel-err ~2e-3 vs the 2e-2 gate) halves store DMA
   23.3us -> 11.65us, which makes the drain ENGINE-bound: every einsum2
   group must cross PSUM->SBUF on DVE or ACT (gpsimd has no PSUM port,
   DMA cannot read PSUM, PE cannot read PSUM).

2. einsum2 groups accumulate in BF16 PSUM, 1024 wide. Consequences:
   - a group is ONE PSUM bank, so the shared staging tag rotates 4 deep
     (vs 2 with f32 groups) and the fill->drain->refill cycle pipelines;
   - DVE epilogues hit the 2x_1p fast mode (all operands 2-byte):
     ~818ns per 1024 instead of 1352;
   - precision: with gamma=0 the group holds exactly bf16(x) (identity
     matmul of bf16 x, no rounding loss); nonzero gamma adds bf16
     rounding of the attention part, well inside the 2e-1 regime.

   Route 'D' groups: one DVE tensor_add (bf16 PSUM + vb -> vb, 2x mode).
   Route 'A' groups: PE identity-matmul folds +x into PSUM (~245ns),
   then one bare ACT copy PSUM -> vb.

Other structure:
   - the energy softmax prologues (row-min DVE, exp ACT with Z-accum,
     dsc = gamma*I/Z) are emitted two rows AHEAD of the einsum2 drain so
     both engines always have independent queued work;
   - all einsum1 tail chunks run row-0-only inline during the loads
     (KPE chunks run all 4 rows), so row 0 closes right after the last
     load; rows 1-3 finish in phase 2 (PE, overlapped with softmax 0);
   - PSUM->vbT transpose staging moves in PAIRS of chunks (one
     1024-wide copy per two chunks) alternating DVE/ACT;
   - gpsimd carries the fp8 casts of v (einsum2 rhs), a few on ACT;
   - PSUM: 4 banks energy + 4 rotating 2KB slots shared by transpose
     pairs, bf16 einsum2 groups and the PT staging tile.
"""

import os
from contextlib import ExitStack

import numpy as np

import concourse.bass as bass
import concourse.tile as tile
from concourse import bacc, mybir
from concourse.bass_utils import run_bass_kernel_spmd
from concourse.masks import make_identity

N_CORES = 8
C = 512
S = 4096
P = 128
CI = C // P  # 4 c-chunks
KD = S // P  # 32 s-chunks of 128
OW = 1024    # einsum2 output group width (1 bf16 PSUM bank)
OG = S // OW  # 4 groups per row

STRIPES = [512] * 7 + [384, 128]
KPE = 18        # chunks whose einsum1 runs all 4 rows inline during loads
TAILF32 = 6     # trailing chunks transposed from xf in f32 (no vb cast dep)
SINGLE_ENG = list("AADADA")  # copy engine per tail chunk (KD-TAILF32..KD-1)
# einsum2 route per (row, group): 'D' = DVE add (PSUM+xf->vb), 'A' = PE
# identity-add + ACT copy. g3 must be 'D': its columns overlap the TAILF32
# region, which never gets a bf16 cast into vb.
ROUTES = (
    ("D", "A", "A", "D"),
    ("A", "D", "A", "D"),
    ("A", "D", "A", "D"),
    ("A", "A", "D", "D"),
)
# engine for each vbT pair copy: 'D' (DVE) / 'A' (ACT)
PAIR_ENG = list("DADADADADADADADA")
# engine for each of the 36 (stripe, ci) fp8 casts: gpsimd with some ACT
VB8_ENG = list("GGGGGGGGGGGGGGGGAGGGAGGGAGGGAGGGAGGG")

F32 = mybir.dt.float32
BF16 = mybir.dt.bfloat16
FP8 = mybir.dt.float8e4


def _body(ctx: ExitStack, tc: tile.TileContext, out: bass.AP, x: bass.AP,
          gamma: bass.AP):
    nc = tc.nc

    persist = ctx.enter_context(tc.tile_pool(name="persist", bufs=1))
    xf = persist.tile([P, CI, S], F32, name="xf")
    vb = persist.tile([P, CI, S], BF16, name="vb")   # bf16 x, then output
    vb8 = persist.tile([P, CI, S], FP8, name="vb8")
    vbT = persist.tile([P, KD, C], BF16, name="vbT")
    p_sb = persist.tile([P, CI, C], BF16, name="p_sb")
    pt_sb = persist.tile([P, CI, C], FP8, name="pt_sb")
    ident = persist.tile([P, P], BF16, name="ident")
    identf = persist.tile([P, P], F32, name="identf")
    identg = persist.tile([P, P], BF16, name="identg")
    gamma_sb = persist.tile([P, 1], F32, name="gamma_sb")
    mn = persist.tile([P, CI], F32, name="mn")
    zsum = persist.tile([P, CI], F32, name="zsum")
    msc = persist.tile([P, CI], F32, name="msc")

    make_identity(nc, ident)
    make_identity(nc, identf)

    x3 = x.rearrange("(ci p) s -> p ci s", p=P)
    out3 = out.rearrange("(ci p) s -> p ci s", p=P)

    epool = ctx.enter_context(tc.tile_pool(name="epool", bufs=4, space="PSUM"))
    e_ps = [epool.tile([P, C], F32, name=f"e{ci}", tag="et") for ci in range(CI)]
    # 2 rotating 4KB slots shared (by tag) between transpose staging pairs,
    # f32 einsum2 groups and the PT staging tile.
    opool = ctx.enter_context(tc.tile_pool(name="opool", bufs=2, space="PSUM"))
    trpool = ctx.enter_context(tc.tile_pool(name="trp", bufs=3))
    dscpool = ctx.enter_context(tc.tile_pool(name="dscp", bufs=2))

    # ---- load + cast + PE-transpose + inline einsum1 ----
    def emit_e1(k):
        rows = range(CI) if k < KPE else (0,)
        for ci in rows:
            nc.tensor.matmul(
                e_ps[ci][:, ci * P:],
                lhsT=vbT[:, k, ci * P:(ci + 1) * P],
                rhs=vbT[:, k, ci * P:],
                start=(k == 0),
                stop=(k == KD - 1),
            )

    tp_cur = None
    col = 0
    for si, w in enumerate(STRIPES):
        sl = slice(col, col + w)
        for ci in range(CI):
            nc.sync.dma_start(out=xf[:, ci, sl], in_=x3[:, ci, sl])
            cast_hi = min(col + w, (KD - TAILF32) * P)
            if col < cast_hi:
                with tc.high_priority():
                    nc.vector.tensor_copy(out=vb[:, ci, col:cast_hi],
                                          in_=xf[:, ci, col:cast_hi])
            eng = VB8_ENG[si * CI + ci]
            if eng == "A":
                nc.scalar.copy(out=vb8[:, ci, sl], in_=xf[:, ci, sl])
            else:
                nc.gpsimd.tensor_copy(out=vb8[:, ci, sl], in_=xf[:, ci, sl])
        if si == 0:
            nc.sync.dma_start(out=gamma_sb[:, :],
                              in_=gamma.to_broadcast((P, 1)))
            # gamma*I once; dsc = identg * (1/Z) per row
            nc.vector.tensor_scalar(
                out=identg[:, :], in0=ident[:, :], scalar1=gamma_sb[:, :],
                scalar2=None, op0=mybir.AluOpType.mult,
            )
        for k in range(col // P, (col + w) // P):
            tailf = k >= KD - TAILF32
            j, half = k // 2, k % 2
            if tailf:
                # single-chunk staging: 2KB tiles ping-pong through the
                # 2-slot rotation at ~700ns/chunk instead of ~1.9us/pair
                tp_cur = opool.tile([P, C], F32, name="tps", tag="op")
                for ci in range(CI):
                    nc.tensor.transpose(
                        out=tp_cur[:, ci * P:(ci + 1) * P],
                        in_=xf[:, ci, k * P:(k + 1) * P],
                        identity=identf[:, :],
                    )
                if SINGLE_ENG[k - (KD - TAILF32)] == "A":
                    nc.scalar.copy(out=vbT[:, k, :], in_=tp_cur[:, :])
                else:
                    nc.vector.tensor_copy(out=vbT[:, k, :], in_=tp_cur[:, :])
                emit_e1(k - 2)
                continue
            if half == 0:
                tp_cur = opool.tile([P, 2, C], F32 if tailf else BF16,
                                    name="tp", tag="op")
            for ci in range(CI):
                nc.tensor.transpose(
                    out=tp_cur[:, half, ci * P:(ci + 1) * P],
                    in_=(xf if tailf else vb)[:, ci, k * P:(k + 1) * P],
                    identity=(identf if tailf else ident)[:, :],
                )
            if half == 1:
                # one 1024-wide PSUM->SBUF copy moves the whole pair
                if PAIR_ENG[j] == "D":
                    nc.vector.tensor_copy(out=vbT[:, 2 * j:2 * j + 2, :],
                                          in_=tp_cur[:, :, :])
                else:
                    nc.scalar.copy(out=vbT[:, 2 * j:2 * j + 2, :],
                                   in_=tp_cur[:, :, :])
                # einsum1 for the PREVIOUS pair (its copy landed during
                # this pair's transposes -- the in-order PE never stalls
                # on a copy it just triggered)
                if j > 0:
                    emit_e1(2 * (j - 1))
                    emit_e1(2 * (j - 1) + 1)
        col += w
    emit_e1(KD - 2)
    emit_e1(KD - 1)

    # ---- phase 2 (close rows 1-3), reconstruct, softmax, einsum2 ----
    def phase2(ci):
        for k in range(KPE, KD):
            nc.tensor.matmul(
                e_ps[ci][:, ci * P:],
                lhsT=vbT[:, k, ci * P:(ci + 1) * P],
                rhs=vbT[:, k, ci * P:],
                start=False,
                stop=(k == KD - 1),
            )

    def emit_tr(ci):
        """Lower-triangle reconstruct for row ci (all its upper sources
        are closed). Emitted as early as possible so the source banks die
        early enough for PT staging to reuse them."""
        for cj in range(ci):
            tr_sb = trpool.tile([P, P], F32, name="tr_sb", tag="tr")
            if (ci + cj) % 2 == 0:
                nc.scalar.copy(out=tr_sb[:, :],
                               in_=e_ps[cj][:, ci * P:(ci + 1) * P])
            else:
                nc.vector.tensor_copy(out=tr_sb[:, :],
                                      in_=e_ps[cj][:, ci * P:(ci + 1) * P])
            nc.tensor.matmul(
                e_ps[ci][:, cj * P:(cj + 1) * P],
                lhsT=tr_sb[:, :],
                rhs=identf[:, :],
                is_transpose=True,
                skip_group_check=True,
            )

    def prologue(ci):
        """Row softmax through dsc. Emitted ahead of the drain so
        DVE/ACT always have queued work."""
        nc.vector.tensor_reduce(
            out=mn[:, ci:ci + 1], in_=e_ps[ci][:, :],
            axis=mybir.AxisListType.X, op=mybir.AluOpType.min,
        )
        nc.scalar.activation(
            out=p_sb[:, ci, :], in_=e_ps[ci][:, :],
            func=mybir.ActivationFunctionType.Exp,
            bias=mn[:, ci:ci + 1], scale=-1.0,
            accum_out=zsum[:, ci:ci + 1],
        )
        nc.vector.reciprocal(out=msc[:, ci:ci + 1], in_=zsum[:, ci:ci + 1])
        dsc = dscpool.tile([P, P], BF16, name="dsc", tag="dsc")
        nc.vector.tensor_scalar(
            out=dsc[:, :], in0=identg[:, :], scalar1=msc[:, ci:ci + 1],
            scalar2=None, op0=mybir.AluOpType.mult,
        )
        return dsc

    def emit_pt(ci, dsc, bank=None):
        # PT block = P_block^T @ diag(gamma/Z) (regular matmul; the
        # transpose datapath ignores rhs values so the scale must go
        # through the normal path). PT for rows 1-3 writes into a DEAD
        # energy bank (all its readers ran), keeping the op-slot FIFO
        # free for einsum2 groups; only row 0's PT takes a FIFO turn.
        if bank is None:
            pt_ps = opool.tile([P, CI, P], F32, name="pt_ps", tag="op")
        else:
            pt_ps = bank.rearrange("p (dj q) -> p dj q", dj=CI)
        for dj in range(CI):
            nc.tensor.matmul(
                pt_ps[:, dj, :],
                lhsT=p_sb[:, ci, dj * P:(dj + 1) * P],
                rhs=dsc[:, :],
                skip_group_check=bank is not None,
            )
        if ci % 2 == 0:
            nc.scalar.copy(out=pt_sb[:, :, ci * P:(ci + 1) * P],
                           in_=pt_ps[:, :, :])
        else:
            nc.vector.tensor_copy(out=pt_sb[:, :, ci * P:(ci + 1) * P],
                                  in_=pt_ps[:, :, :])

    def emit_groups(ci, gs):
        for g in gs:
            lo = g * OW
            route = ROUTES[ci][g]
            o_ps = opool.tile([P, OW], F32, name="o_ps", tag="op")
            for hb in range(OW // 512):
                hlo = lo + hb * 512
                hsl = slice(hlo, hlo + 512)
                psl = slice(hb * 512, hb * 512 + 512)
                for h in range(CI // 2):
                    nc.tensor.matmul(
                        o_ps[:, psl],
                        lhsT=pt_sb[:, 2 * h:2 * h + 2, ci * P:(ci + 1) * P],
                        rhs=vb8[:, 2 * h:2 * h + 2, hsl],
                        start=(h == 0),
                        stop=(h == CI // 2 - 1 and route == "D"),
                        perf_mode=mybir.MatmulPerfMode.DoubleRow,
                    )
                if route == "A":
                    # fold +x on the PE so the drain is a bare ACT copy
                    nc.tensor.matmul(
                        o_ps[:, psl],
                        lhsT=ident[:, :],
                        rhs=vb[:, ci, hsl],
                        start=False,
                        stop=True,
                    )
            gsl = slice(lo, lo + OW)
            if route == "D":
                nc.vector.tensor_add(out=vb[:, ci, gsl], in0=o_ps[:, :],
                                     in1=xf[:, ci, gsl])
            else:
                nc.scalar.copy(out=vb[:, ci, gsl], in_=o_ps[:, :])
            nc.sync.dma_start(out=out3[:, ci, gsl], in_=vb[:, ci, gsl])

    # Interleaved drain. Emission order IS the static per-engine program
    # order, so: row 0's softmax chain is emitted before phase2 (else the
    # Tile scheduler interleaves rows 1-3 einsum1 into the critical tail),
    # and each row's PT staging tile is emitted right after the previous
    # row's first two groups so the 2-slot PSUM rotation never stalls the
    # row hand-off.
    dsc0 = prologue(0)
    phase2(1)
    emit_tr(1)
    dsc1 = prologue(1)
    emit_pt(0, dsc0)
    emit_groups(0, (0, 1))
    phase2(2)
    emit_tr(2)
    dsc2 = prologue(2)
    phase2(3)
    emit_tr(3)
    # e0's readers (exp0 + all tr reads) are done: PT1 reuses its bank
    emit_pt(1, dsc1, bank=e_ps[0])
    emit_groups(0, (2, 3))
    emit_groups(1, (0, 1))
    dsc3 = prologue(3)
    emit_pt(2, dsc2, bank=e_ps[1])
    emit_groups(1, (2, 3))
    emit_groups(2, (0, 1))
    emit_pt(3, dsc3, bank=e_ps[2])
    emit_groups(2, (2, 3))
    emit_groups(3, (0, 1, 2, 3))


def build():
    nc = bacc.Bacc("TRN2", target_bir_lowering=False, debug=False,
                   num_devices=N_CORES)
    x = nc.dram_tensor("x", [C, S], F32, kind="ExternalInput")
    gamma = nc.dram_tensor("gamma", [1], F32, kind="ExternalInput")
    out = nc.dram_tensor("out", [C, S], BF16, kind="ExternalOutput")
    with tile.TileContext(nc) as tc:
        with ExitStack() as ctx:
            _body(ctx, tc, out.ap(), x.ap(), gamma.ap())
    nc.compile()
    return nc


_NC_CACHE = {}
LAST_RESULTS = None


def kernel(x: np.ndarray, gamma: np.ndarray) -> np.ndarray:
    global LAST_RESULTS
    x = np.ascontiguousarray(np.asarray(x, dtype=np.float32))
    gamma = np.ascontiguousarray(np.asarray(gamma, dtype=np.float32))
    n, c, h, w = x.shape
    assert (n, c, h * w) == (N_CORES, C, S), f"unexpected shape {x.shape}"

    os.environ["BASS_NEVER_TRACE"] = "1"

    if "nc" not in _NC_CACHE:
        _NC_CACHE["nc"] = build()
    nc = _NC_CACHE["nc"]

    in_maps = [
        {"x": x[i].reshape(C, S), "gamma": gamma} for i in range(N_CORES)
    ]
    res = run_bass_kernel_spmd(nc, in_maps, core_ids=list(range(N_CORES)))
    LAST_RESULTS = res
    out = np.stack(
        [np.asarray(res.results[i]["out"]) for i in range(N_CORES)], axis=0
    ).astype(np.float32)
    return out.reshape(n, c, h, w)


if __name__ == "__main__":
    xs = np.random.randn(N_CORES, C, 64, 64).astype(np.float32)
    g = np.zeros((1,), np.float32)
    o = kernel(xs, g)
    print("ok", o.shape, np.abs(o - xs).max())


# revision 19
# speedup vs baseline: 1.0454x; 1.0123x over previous
"""Channel-attention (CAM) Trainium2 Bass kernel.

Reference computation (per batch n):
    v = x[n].reshape(C, S)                 # C=512, S=H*W=4096
    energy = v @ v.T                       # (C, C)
    att = softmax(max_row(energy) - energy, axis=-1)
        = exp(min_row(energy) - energy) / Z
    out[n] = gamma * (att @ v) + x[n]

Sharding: data-parallel over N=8 batches across 8 NeuronCores; each core
computes one full C x C attention locally (no collectives).

v3 design (baseline 61377ns -> target ~41us). Two structural changes:

1. BF16 output (rel-err ~2e-3 vs the 2e-2 gate) halves store DMA
   23.3us -> 11.65us, which makes the drain ENGINE-bound: every einsum2
   group must cross PSUM->SBUF on DVE or ACT (gpsimd has no PSUM port,
   DMA cannot read PSUM, PE cannot read PSUM).

2. einsum2 groups accumulate in BF16 PSUM, 1024 wide. Consequences:
   - a group is ONE PSUM bank, so the shared staging tag rotates 4 deep
     (vs 2 with f32 groups) and the fill->drain->refill cycle pipelines;
   - DVE epilogues hit the 2x_1p fast mode (all operands 2-byte):
     ~818ns per 1024 instead of 1352;
   - precision: with gamma=0 the group holds exactly bf16(x) (identity
     matmul of bf16 x, no rounding loss); nonzero gamma adds bf16
     rounding of the attention part, well inside the 2e-1 regime.

   Route 'D' groups: one DVE tensor_add (bf16 PSUM + vb -> vb, 2x mode).
   Route 'A' groups: PE identity-matmul folds +x into PSUM (~245ns),
   then one bare ACT copy PSUM -> vb.

Other structure:
   - the energy softmax prologues (row-min DVE, exp ACT with Z-accum,
     dsc = gamma*I/Z) are emitted two rows AHEAD of the einsum2 drain so
     both engines always have independent queued work;
   - all einsum1 tail chunks run row-0-only inline during the loads
     (KPE chunks run all 4 rows), so row 0 closes right after the last
     load; rows 1-3 finish in phase 2 (PE, overlapped with softmax 0);
   - PSUM->vbT transpose staging moves in PAIRS of chunks (one
     1024-wide copy per two chunks) alternating DVE/ACT;
   - gpsimd carries the fp8 casts of v (einsum2 rhs), a few on ACT;
   - PSUM: 4 banks energy + 4 rotating 2KB slots shared by transpose
     pairs, bf16 einsum2 groups and the PT staging tile.
"""

import os
from contextlib import ExitStack

import numpy as np

import concourse.bass as bass
import concourse.tile as tile
from concourse import bacc, mybir
from concourse.bass_utils import run_bass_kernel_spmd
from concourse.masks import make_identity

N_CORES = 8
C = 512
S = 4096
P = 128
CI = C // P  # 4 c-chunks
KD = S // P  # 32 s-chunks of 128
OW = 1024    # einsum2 output group width (1 bf16 PSUM bank)
OG = S // OW  # 4 groups per row

STRIPES = [512] * 7 + [384, 128]
KPE = 18        # chunks whose einsum1 runs all 4 rows inline during loads
TAILF32 = 6     # trailing chunks transposed from xf in f32 (no vb cast dep)
SINGLE_ENG = list("AADADA")  # copy engine per tail chunk (KD-TAILF32..KD-1)
# einsum2 route per (row, group): 'D' = DVE add (PSUM+xf->vb), 'A' = PE
# identity-add + ACT copy. g3 must be 'D': its columns overlap the TAILF32
# region, which never gets a bf16 cast into vb.
ROUTES = (
    ("A", "D", "A", "D"),
    ("A", "D", "A", "D"),
    ("A", "D", "A", "D"),
    ("A", "A", "D", "D"),
)
# engine for each vbT pair copy: 'D' (DVE) / 'A' (ACT)
PAIR_ENG = list("DADADADADADADADA")
# engine for each of the 36 (stripe, ci) fp8 casts: gpsimd with some ACT
VB8_ENG = list("GGGGGGGGGGGGGGGGAGGGAGGGAGGGAGGGAGGG")

F32 = mybir.dt.float32
BF16 = mybir.dt.bfloat16
FP8 = mybir.dt.float8e4


def _body(ctx: ExitStack, tc: tile.TileContext, out: bass.AP, x: bass.AP,
          gamma: bass.AP):
    nc = tc.nc

    persist = ctx.enter_context(tc.tile_pool(name="persist", bufs=1))
    xf = persist.tile([P, CI, S], F32, name="xf")
    vb = persist.tile([P, CI, S], BF16, name="vb")   # bf16 x, then output
    vb8 = persist.tile([P, CI, S], FP8, name="vb8")
    vbT = persist.tile([P, KD, C], BF16, name="vbT")
    p_sb = persist.tile([P, CI, C], BF16, name="p_sb")
    pt_sb = persist.tile([P, CI, C], FP8, name="pt_sb")
    ident = persist.tile([P, P], BF16, name="ident")
    identf = persist.tile([P, P], F32, name="identf")
    identg = persist.tile([P, P], BF16, name="identg")
    gamma_sb = persist.tile([P, 1], F32, name="gamma_sb")
    mn = persist.tile([P, CI], F32, name="mn")
    zsum = persist.tile([P, CI], F32, name="zsum")
    msc = persist.tile([P, CI], F32, name="msc")

    make_identity(nc, ident)
    make_identity(nc, identf)

    x3 = x.rearrange("(ci p) s -> p ci s", p=P)
    out3 = out.rearrange("(ci p) s -> p ci s", p=P)

    epool = ctx.enter_context(tc.tile_pool(name="epool", bufs=4, space="PSUM"))
    e_ps = [epool.tile([P, C], F32, name=f"e{ci}", tag="et") for ci in range(CI)]
    # 2 rotating 4KB slots shared (by tag) between transpose staging pairs,
    # f32 einsum2 groups and the PT staging tile.
    opool = ctx.enter_context(tc.tile_pool(name="opool", bufs=2, space="PSUM"))
    trpool = ctx.enter_context(tc.tile_pool(name="trp", bufs=3))
    dscpool = ctx.enter_context(tc.tile_pool(name="dscp", bufs=2))

    # ---- load + cast + PE-transpose + inline einsum1 ----
    def emit_e1(k):
        rows = range(CI) if k < KPE else (0,)
        for ci in rows:
            nc.tensor.matmul(
                e_ps[ci][:, ci * P:],
                lhsT=vbT[:, k, ci * P:(ci + 1) * P],
                rhs=vbT[:, k, ci * P:],
                start=(k == 0),
                stop=(k == KD - 1),
            )

    tp_cur = None
    col = 0
    for si, w in enumerate(STRIPES):
        sl = slice(col, col + w)
        for ci in range(CI):
            nc.sync.dma_start(out=xf[:, ci, sl], in_=x3[:, ci, sl])
            cast_hi = min(col + w, (KD - TAILF32) * P)
            if col < cast_hi:
                with tc.high_priority():
                    nc.vector.tensor_copy(out=vb[:, ci, col:cast_hi],
                                          in_=xf[:, ci, col:cast_hi])
            eng = VB8_ENG[si * CI + ci]
            if eng == "A":
                nc.scalar.copy(out=vb8[:, ci, sl], in_=xf[:, ci, sl])
            else:
                nc.gpsimd.tensor_copy(out=vb8[:, ci, sl], in_=xf[:, ci, sl])
        if si == 0:
            nc.sync.dma_start(out=gamma_sb[:, :],
                              in_=gamma.to_broadcast((P, 1)))
            # gamma*I once; dsc = identg * (1/Z) per row
            nc.vector.tensor_scalar(
                out=identg[:, :], in0=ident[:, :], scalar1=gamma_sb[:, :],
                scalar2=None, op0=mybir.AluOpType.mult,
            )
        for k in range(col // P, (col + w) // P):
            tailf = k >= KD - TAILF32
            j, half = k // 2, k % 2
            if tailf:
                # single-chunk staging: 2KB tiles ping-pong through the
                # 2-slot rotation at ~700ns/chunk instead of ~1.9us/pair
                tp_cur = opool.tile([P, C], F32, name="tps", tag="op")
                for ci in range(CI):
                    nc.tensor.transpose(
                        out=tp_cur[:, ci * P:(ci + 1) * P],
                        in_=xf[:, ci, k * P:(k + 1) * P],
                        identity=identf[:, :],
                    )
                if SINGLE_ENG[k - (KD - TAILF32)] == "A":
                    nc.scalar.copy(out=vbT[:, k, :], in_=tp_cur[:, :])
                else:
                    nc.vector.tensor_copy(out=vbT[:, k, :], in_=tp_cur[:, :])
                emit_e1(k - 2)
                continue
            if half == 0:
                tp_cur = opool.tile([P, 2, C], F32 if tailf else BF16,
                                    name="tp", tag="op")
            for ci in range(CI):
                nc.tensor.transpose(
                    out=tp_cur[:, half, ci * P:(ci + 1) * P],
                    in_=(xf if tailf else vb)[:, ci, k * P:(k + 1) * P],
                    identity=(identf if tailf else ident)[:, :],
                )
            if half == 1:
                # one 1024-wide PSUM->SBUF copy moves the whole pair
                if PAIR_ENG[j] == "D":
                    nc.vector.tensor_copy(out=vbT[:, 2 * j:2 * j + 2, :],
                                          in_=tp_cur[:, :, :])
                else:
                    nc.scalar.copy(out=vbT[:, 2 * j:2 * j + 2, :],
                                   in_=tp_cur[:, :, :])
                # einsum1 for the PREVIOUS pair (its copy landed during
                # this pair's transposes -- the in-order PE never stalls
                # on a copy it just triggered)
                if j > 0:
                    emit_e1(2 * (j - 1))
                    emit_e1(2 * (j - 1) + 1)
        col += w
    emit_e1(KD - 2)
    emit_e1(KD - 1)

    # ---- phase 2 (close rows 1-3), reconstruct, softmax, einsum2 ----
    def phase2(ci):
        for k in range(KPE, KD):
            nc.tensor.matmul(
                e_ps[ci][:, ci * P:],
                lhsT=vbT[:, k, ci * P:(ci + 1) * P],
                rhs=vbT[:, k, ci * P:],
                start=False,
                stop=(k == KD - 1),
            )

    def emit_tr(ci):
        """Lower-triangle reconstruct for row ci (all its upper sources
        are closed). Emitted as early as possible so the source banks die
        early enough for PT staging to reuse them."""
        for cj in range(ci):
            tr_sb = trpool.tile([P, P], F32, name="tr_sb", tag="tr")
            if (ci + cj) % 2 == 0:
                nc.scalar.copy(out=tr_sb[:, :],
                               in_=e_ps[cj][:, ci * P:(ci + 1) * P])
            else:
                nc.vector.tensor_copy(out=tr_sb[:, :],
                                      in_=e_ps[cj][:, ci * P:(ci + 1) * P])
            nc.tensor.matmul(
                e_ps[ci][:, cj * P:(cj + 1) * P],
                lhsT=tr_sb[:, :],
                rhs=identf[:, :],
                is_transpose=True,
                skip_group_check=True,
            )

    def prologue(ci):
        """Row softmax through dsc. Emitted ahead of the drain so
        DVE/ACT always have queued work."""
        nc.vector.tensor_reduce(
            out=mn[:, ci:ci + 1], in_=e_ps[ci][:, :],
            axis=mybir.AxisListType.X, op=mybir.AluOpType.min,
        )
        nc.scalar.activation(
            out=p_sb[:, ci, :], in_=e_ps[ci][:, :],
            func=mybir.ActivationFunctionType.Exp,
            bias=mn[:, ci:ci + 1], scale=-1.0,
            accum_out=zsum[:, ci:ci + 1],
        )
        nc.vector.reciprocal(out=msc[:, ci:ci + 1], in_=zsum[:, ci:ci + 1])
        dsc = dscpool.tile([P, P], BF16, name="dsc", tag="dsc")
        nc.vector.tensor_scalar(
            out=dsc[:, :], in0=identg[:, :], scalar1=msc[:, ci:ci + 1],
            scalar2=None, op0=mybir.AluOpType.mult,
        )
        return dsc

    def emit_pt(ci, dsc, bank=None):
        # PT block = P_block^T @ diag(gamma/Z) (regular matmul; the
        # transpose datapath ignores rhs values so the scale must go
        # through the normal path). PT for rows 1-3 writes into a DEAD
        # energy bank (all its readers ran), keeping the op-slot FIFO
        # free for einsum2 groups; only row 0's PT takes a FIFO turn.
        if bank is None:
            pt_ps = opool.tile([P, CI, P], F32, name="pt_ps", tag="op")
        else:
            pt_ps = bank.rearrange("p (dj q) -> p dj q", dj=CI)
        for dj in range(CI):
            nc.tensor.matmul(
                pt_ps[:, dj, :],
                lhsT=p_sb[:, ci, dj * P:(dj + 1) * P],
                rhs=dsc[:, :],
                skip_group_check=bank is not None,
            )
        if ci % 2 == 0:
            nc.scalar.copy(out=pt_sb[:, :, ci * P:(ci + 1) * P],
                           in_=pt_ps[:, :, :])
        else:
            nc.vector.tensor_copy(out=pt_sb[:, :, ci * P:(ci + 1) * P],
                                  in_=pt_ps[:, :, :])

    def emit_groups(ci, gs):
        for g in gs:
            lo = g * OW
            route = ROUTES[ci][g]
            o_ps = opool.tile([P, OW], F32, name="o_ps", tag="op")
            for hb in range(OW // 512):
                hlo = lo + hb * 512
                hsl = slice(hlo, hlo + 512)
                psl = slice(hb * 512, hb * 512 + 512)
                for h in range(CI // 2):
                    nc.tensor.matmul(
                        o_ps[:, psl],
                        lhsT=pt_sb[:, 2 * h:2 * h + 2, ci * P:(ci + 1) * P],
                        rhs=vb8[:, 2 * h:2 * h + 2, hsl],
                        start=(h == 0),
                        stop=(h == CI // 2 - 1 and route == "D"),
                        perf_mode=mybir.MatmulPerfMode.DoubleRow,
                    )
                if route == "A":
                    # fold +x on the PE so the drain is a bare ACT copy
                    nc.tensor.matmul(
                        o_ps[:, psl],
                        lhsT=ident[:, :],
                        rhs=vb[:, ci, hsl],
                        start=False,
                        stop=True,
                    )
            gsl = slice(lo, lo + OW)
            if route == "D":
                nc.vector.tensor_add(out=vb[:, ci, gsl], in0=o_ps[:, :],
                                     in1=xf[:, ci, gsl])
            else:
                nc.scalar.copy(out=vb[:, ci, gsl], in_=o_ps[:, :])
            nc.sync.dma_start(out=out3[:, ci, gsl], in_=vb[:, ci, gsl])

    # Interleaved drain. Emission order IS the static per-engine program
    # order, so: row 0's softmax chain is emitted before phase2 (else the
    # Tile scheduler interleaves rows 1-3 einsum1 into the critical tail),
    # and each row's PT staging tile is emitted right after the previous
    # row's first two groups so the 2-slot PSUM rotation never stalls the
    # row hand-off.
    dsc0 = prologue(0)
    phase2(1)
    emit_tr(1)
    dsc1 = prologue(1)
    emit_pt(0, dsc0)
    emit_groups(0, (0, 1))
    phase2(2)
    emit_tr(2)
    dsc2 = prologue(2)
    phase2(3)
    emit_tr(3)
    # e0's readers (exp0 + all tr reads) are done: PT1 reuses its bank
    emit_pt(1, dsc1, bank=e_ps[0])
    emit_groups(0, (2, 3))
    emit_groups(1, (0, 1))
    dsc3 = prologue(3)
    emit_pt(2, dsc2, bank=e_ps[1])
    emit_groups(1, (2, 3))
    emit_groups(2, (0, 1))
    emit_pt(3, dsc3, bank=e_ps[2])
    emit_groups(2, (2, 3))
    emit_groups(3, (0, 1, 2, 3))


def build():
    nc = bacc.Bacc("TRN2", target_bir_lowering=False, debug=False,
                   num_devices=N_CORES)
    x = nc.dram_tensor("x", [C, S], F32, kind="ExternalInput")
    gamma = nc.dram_tensor("gamma", [1], F32, kind="ExternalInput")
    out = nc.dram_tensor("out", [C, S], BF16, kind="ExternalOutput")
    with tile.TileContext(nc) as tc:
        with ExitStack() as ctx:
            _body(ctx, tc, out.ap(), x.ap(), gamma.ap())
    nc.compile()
    return nc


_NC_CACHE = {}
LAST_RESULTS = None


def kernel(x: np.ndarray, gamma: np.ndarray) -> np.ndarray:
    global LAST_RESULTS
    x = np.ascontiguousarray(np.asarray(x, dtype=np.float32))
    gamma = np.ascontiguousarray(np.asarray(gamma, dtype=np.float32))
    n, c, h, w = x.shape
    assert (n, c, h * w) == (N_CORES, C, S), f"unexpected shape {x.shape}"

    os.environ["BASS_NEVER_TRACE"] = "1"

    if "nc" not in _NC_CACHE:
        _NC_CACHE["nc"] = build()
    nc = _NC_CACHE["nc"]

    in_maps = [
        {"x": x[i].reshape(C, S), "gamma": gamma} for i in range(N_CORES)
    ]
    res = run_bass_kernel_spmd(nc, in_maps, core_ids=list(range(N_CORES)))
    LAST_RESULTS = res
    out = np.stack(
        [np.asarray(res.results[i]["out"]) for i in range(N_CORES)], axis=0
    ).astype(np.float32)
    return out.reshape(n, c, h, w)


if __name__ == "__main__":
    xs = np.random.randn(N_CORES, C, 64, 64).astype(np.float32)
    g = np.zeros((1,), np.float32)
    o = kernel(xs, g)
    print("ok", o.shape, np.abs(o - xs).max())


# revision 29
# speedup vs baseline: 1.0726x; 1.0260x over previous
"""Channel-attention (CAM) Trainium2 Bass kernel.

Reference computation (per batch n):
    v = x[n].reshape(C, S)                 # C=512, S=H*W=4096
    energy = v @ v.T                       # (C, C)
    att = softmax(max_row(energy) - energy, axis=-1)
        = exp(min_row(energy) - energy) / Z
    out[n] = gamma * (att @ v) + x[n]

Sharding: data-parallel over N=8 batches across 8 NeuronCores; each core
computes one full C x C attention locally (no collectives).

v3 design, 55492ns (TimelineSim; prior baseline 61377ns, DMA floor
34.9us). The structural change vs the f32 baseline: the OUTPUT IS
STORED AS BF16 (rel-err ~2.9e-3 at gamma=0, vs the 2e-2 harness gate),
halving store DMA 23.3us -> 11.65us. That makes the back half of the
kernel bound by PSUM->SBUF drain bandwidth instead of DMA: every
einsum2 output group must cross PSUM->SBUF on DVE or ACT (gpsimd has
no PSUM port; DMA and the PE cannot read PSUM), and with the 4 energy
banks resident, only two rotating 4KB PSUM slots remain, so the drain
cycles at (fill + drain)/2 per 1024-wide group (~1.0-1.1us).

Drain routes per 1024-wide group (ROUTES, tuned empirically):
  'D': one DVE tensor_add (f32 PSUM + xf -> bf16 vb), ~1192ns;
  'A': the PE folds +x into the accumulation group with a bf16
       identity matmul (~245ns/512), then one bare ACT copy
       PSUM -> bf16 vb, ~1038ns.
Both engines also carry the fixed softmax work (DVE: row-min from
PSUM, dsc = gamma*I * 1/Z; ACT: exp with Z-accum), balanced via the
*_ENG knobs. Row 0's first group is SPLIT into 512 halves to shorten
the first store's fill+drain+issue chain.

Timeline structure:
  - loads in column stripes; DVE casts x -> bf16 (vb) as stripes land;
    gpsimd casts x -> fp8 (vb8, einsum2 rhs) with a few spilled to ACT;
  - PE transposes chunks into 2 rotating PSUM slots; one 1024-wide
    PSUM->SBUF copy moves each PAIR of chunks to vbT (engines per
    PAIR_ENG); the last TAILF32 chunks transpose from xf in f32
    (no cast dependency) and move as single-chunk copies so the
    load->row0-energy chain stays short;
  - einsum1 runs inline during loads (all 4 rows below KPE, row 0 only
    after), so row 0's energy closes right after the last load; rows
    1-3 close in phase 2 on the PE, overlapped with row 0's softmax;
  - the lower block-triangle is reconstructed via PE transposes as
    early as each row closes (emit_tr), which lets PT staging for rows
    1-3 write into DEAD energy banks instead of taking drain slots;
  - softmax prologues are emitted ahead of the drain; each row's PT
    staging interleaves after the previous row's first two groups.

Remaining gap vs the ~47us ideal (load 23.3 + min-chain ~5 + store
drain ~17): the 2-slot drain cycle and the row-0 energy-close chain;
both are PSUM-capacity-bound (energy rows occupy 4 of 8 banks).
"""

import os
from contextlib import ExitStack, nullcontext as _null

import numpy as np

import concourse.bass as bass
import concourse.tile as tile
from concourse import bacc, mybir
from concourse.bass_utils import run_bass_kernel_spmd
from concourse.masks import make_identity

N_CORES = 8
C = 512
S = 4096
P = 128
CI = C // P  # 4 c-chunks
KD = S // P  # 32 s-chunks of 128
OW = 1024    # einsum2 output group width (1 bf16 PSUM bank)
OG = S // OW  # 4 groups per row

STRIPES = [512] * 7 + [384, 128]
KPE = 18        # chunks whose einsum1 runs all 4 rows inline during loads
TAILF32 = 4     # trailing chunks transposed from xf in f32 (no vb cast dep)
SINGLE_ENG = list("DADA")  # copy engine per tail chunk (KD-TAILF32..KD-1)
PTCOPY_ENG = list("ADAD")  # engine for each row's PT fp8 copy
CAST_ENG = list("DDDDDDDDD")  # bf16-cast engine per stripe
CAST_HP = True
TAIL_PRIO = None
SPLIT_GROUPS = {(0, 0)}  # groups emitted as two 512 halves (latency)
SPLIT_W = 512
# einsum2 route per (row, group): 'D' = DVE add (PSUM+xf->vb), 'A' = PE
# identity-add + ACT copy. Routes touching the TAILF32 columns (g3) must
# be 'D' -- that region never gets a bf16 cast into vb (asserted below).
ROUTES = (
    ("A", "D", "A", "D"),
    ("A", "D", "A", "D"),
    ("A", "A", "D", "D"),
    ("A", "D", "A", "D"),
)
# engine for each vbT pair copy: 'D' (DVE) / 'A' (ACT)
PAIR_ENG = list("DDAADDAADDAADDAA")
# engine for each of the 36 (stripe, ci) fp8 casts: gpsimd with some ACT
VB8_ENG = list("GGGGGGGGGGGGGGGGAGGGAGGGAGGGAGGGAGGG")

F32 = mybir.dt.float32
BF16 = mybir.dt.bfloat16
FP8 = mybir.dt.float8e4


def _body(ctx: ExitStack, tc: tile.TileContext, out: bass.AP, x: bass.AP,
          gamma: bass.AP):
    nc = tc.nc

    persist = ctx.enter_context(tc.tile_pool(name="persist", bufs=1))
    xf = persist.tile([P, CI, S], F32, name="xf")
    vb = persist.tile([P, CI, S], BF16, name="vb")   # bf16 x, then output
    vb8 = persist.tile([P, CI, S], FP8, name="vb8")
    vbT = persist.tile([P, KD, C], BF16, name="vbT")
    p_sb = persist.tile([P, CI, C], BF16, name="p_sb")
    pt_sb = persist.tile([P, CI, C], FP8, name="pt_sb")
    ident = persist.tile([P, P], BF16, name="ident")
    identf = persist.tile([P, P], F32, name="identf")
    identg = persist.tile([P, P], BF16, name="identg")
    gamma_sb = persist.tile([P, 1], F32, name="gamma_sb")
    mn = persist.tile([P, CI], F32, name="mn")
    zsum = persist.tile([P, CI], F32, name="zsum")
    msc = persist.tile([P, CI], F32, name="msc")

    make_identity(nc, ident)
    make_identity(nc, identf)

    x3 = x.rearrange("(ci p) s -> p ci s", p=P)
    out3 = out.rearrange("(ci p) s -> p ci s", p=P)

    epool = ctx.enter_context(tc.tile_pool(name="epool", bufs=4, space="PSUM"))
    e_ps = [epool.tile([P, C], F32, name=f"e{ci}", tag="et") for ci in range(CI)]
    # 2 rotating 4KB slots shared (by tag) between transpose staging
    # pairs/singles, f32 einsum2 groups and row 0's PT staging tile.
    opool = ctx.enter_context(tc.tile_pool(name="opool", bufs=2, space="PSUM"))
    trpool = ctx.enter_context(tc.tile_pool(name="trp", bufs=3))
    dscpool = ctx.enter_context(tc.tile_pool(name="dscp", bufs=2))

    # ---- load + cast + PE-transpose + inline einsum1 ----
    def emit_e1(k):
        rows = range(CI) if k < KPE else (0,)
        for ci in rows:
            nc.tensor.matmul(
                e_ps[ci][:, ci * P:],
                lhsT=vbT[:, k, ci * P:(ci + 1) * P],
                rhs=vbT[:, k, ci * P:],
                start=(k == 0),
                stop=(k == KD - 1),
            )

    tp_cur = None
    col = 0
    for si, w in enumerate(STRIPES):
        sl = slice(col, col + w)
        for ci in range(CI):
            nc.sync.dma_start(out=xf[:, ci, sl], in_=x3[:, ci, sl])
            cast_hi = min(col + w, (KD - TAILF32) * P)
            if col < cast_hi:
                ceng = nc.scalar if CAST_ENG[si] == "A" else nc.vector
                if CAST_HP:
                    with tc.high_priority():
                        if CAST_ENG[si] == "A":
                            ceng.copy(out=vb[:, ci, col:cast_hi],
                                      in_=xf[:, ci, col:cast_hi])
                        else:
                            ceng.tensor_copy(out=vb[:, ci, col:cast_hi],
                                             in_=xf[:, ci, col:cast_hi])
                elif CAST_ENG[si] == "A":
                    ceng.copy(out=vb[:, ci, col:cast_hi],
                              in_=xf[:, ci, col:cast_hi])
                else:
                    ceng.tensor_copy(out=vb[:, ci, col:cast_hi],
                                     in_=xf[:, ci, col:cast_hi])
            eng = VB8_ENG[si * CI + ci]
            if eng == "A":
                nc.scalar.copy(out=vb8[:, ci, sl], in_=xf[:, ci, sl])
            else:
                nc.gpsimd.tensor_copy(out=vb8[:, ci, sl], in_=xf[:, ci, sl])
        if si == 0:
            nc.sync.dma_start(out=gamma_sb[:, :],
                              in_=gamma.to_broadcast((P, 1)))
            # gamma*I once; dsc = identg * (1/Z) per row
            nc.vector.tensor_scalar(
                out=identg[:, :], in0=ident[:, :], scalar1=gamma_sb[:, :],
                scalar2=None, op0=mybir.AluOpType.mult,
            )
        for k in range(col // P, (col + w) // P):
            tailf = k >= KD - TAILF32
            j, half = k // 2, k % 2
            if tailf:
                # single-chunk staging: 2KB tiles ping-pong through the
                # 2-slot rotation at ~700ns/chunk instead of ~1.9us/pair
                with tc.high_priority(offset=TAIL_PRIO) if TAIL_PRIO else _null():
                    tp_cur = opool.tile([P, C], F32, name="tps", tag="op")
                    for ci in range(CI):
                        nc.tensor.transpose(
                            out=tp_cur[:, ci * P:(ci + 1) * P],
                            in_=xf[:, ci, k * P:(k + 1) * P],
                            identity=identf[:, :],
                        )
                    if SINGLE_ENG[k - (KD - TAILF32)] == "A":
                        nc.scalar.copy(out=vbT[:, k, :], in_=tp_cur[:, :])
                    else:
                        nc.vector.tensor_copy(out=vbT[:, k, :], in_=tp_cur[:, :])
                    emit_e1(k - 2)
                continue
            if half == 0:
                tp_cur = opool.tile([P, 2, C], F32 if tailf else BF16,
                                    name="tp", tag="op")
            for ci in range(CI):
                nc.tensor.transpose(
                    out=tp_cur[:, half, ci * P:(ci + 1) * P],
                    in_=(xf if tailf else vb)[:, ci, k * P:(k + 1) * P],
                    identity=(identf if tailf else ident)[:, :],
                )
            if half == 1:
                # one 1024-wide PSUM->SBUF copy moves the whole pair
                if PAIR_ENG[j] == "D":
                    nc.vector.tensor_copy(out=vbT[:, 2 * j:2 * j + 2, :],
                                          in_=tp_cur[:, :, :])
                else:
                    nc.scalar.copy(out=vbT[:, 2 * j:2 * j + 2, :],
                                   in_=tp_cur[:, :, :])
                # einsum1 for the PREVIOUS pair (its copy landed during
                # this pair's transposes -- the in-order PE never stalls
                # on a copy it just triggered)
                if j > 0:
                    emit_e1(2 * (j - 1))
                    emit_e1(2 * (j - 1) + 1)
        col += w
    emit_e1(KD - 2)
    emit_e1(KD - 1)

    # ---- phase 2 (close rows 1-3), reconstruct, softmax, einsum2 ----
    def phase2(ci):
        for k in range(KPE, KD):
            nc.tensor.matmul(
                e_ps[ci][:, ci * P:],
                lhsT=vbT[:, k, ci * P:(ci + 1) * P],
                rhs=vbT[:, k, ci * P:],
                start=False,
                stop=(k == KD - 1),
            )

    def emit_tr(ci):
        """Lower-triangle reconstruct for row ci (all its upper sources
        are closed). Emitted as early as possible so the source banks die
        early enough for PT staging to reuse them."""
        for cj in range(ci):
            tr_sb = trpool.tile([P, P], F32, name="tr_sb", tag="tr")
            if (ci + cj) % 2 == 0:
                nc.scalar.copy(out=tr_sb[:, :],
                               in_=e_ps[cj][:, ci * P:(ci + 1) * P])
            else:
                nc.vector.tensor_copy(out=tr_sb[:, :],
                                      in_=e_ps[cj][:, ci * P:(ci + 1) * P])
            nc.tensor.matmul(
                e_ps[ci][:, cj * P:(cj + 1) * P],
                lhsT=tr_sb[:, :],
                rhs=identf[:, :],
                is_transpose=True,
                skip_group_check=True,
            )

    def prologue(ci):
        """Row softmax through dsc. Emitted ahead of the drain so
        DVE/ACT always have queued work."""
        nc.vector.tensor_reduce(
            out=mn[:, ci:ci + 1], in_=e_ps[ci][:, :],
            axis=mybir.AxisListType.X, op=mybir.AluOpType.min,
        )
        nc.scalar.activation(
            out=p_sb[:, ci, :], in_=e_ps[ci][:, :],
            func=mybir.ActivationFunctionType.Exp,
            bias=mn[:, ci:ci + 1], scale=-1.0,
            accum_out=zsum[:, ci:ci + 1],
        )
        nc.vector.reciprocal(out=msc[:, ci:ci + 1], in_=zsum[:, ci:ci + 1])
        dsc = dscpool.tile([P, P], BF16, name="dsc", tag="dsc")
        nc.vector.tensor_scalar(
            out=dsc[:, :], in0=identg[:, :], scalar1=msc[:, ci:ci + 1],
            scalar2=None, op0=mybir.AluOpType.mult,
        )
        return dsc

    def emit_pt(ci, dsc, bank=None):
        # PT block = P_block^T @ diag(gamma/Z) (regular matmul; the
        # transpose datapath ignores rhs values so the scale must go
        # through the normal path). PT for rows 1-3 writes into a DEAD
        # energy bank (all its readers ran), keeping the op-slot FIFO
        # free for einsum2 groups; only row 0's PT takes a FIFO turn.
        if bank is None:
            pt_ps = opool.tile([P, CI, P], F32, name="pt_ps", tag="op")
        else:
            pt_ps = bank.rearrange("p (dj q) -> p dj q", dj=CI)
        for dj in range(CI):
            nc.tensor.matmul(
                pt_ps[:, dj, :],
                lhsT=p_sb[:, ci, dj * P:(dj + 1) * P],
                rhs=dsc[:, :],
                skip_group_check=bank is not None,
            )
        if PTCOPY_ENG[ci] == "A":
            nc.scalar.copy(out=pt_sb[:, :, ci * P:(ci + 1) * P],
                           in_=pt_ps[:, :, :])
        else:
            nc.vector.tensor_copy(out=pt_sb[:, :, ci * P:(ci + 1) * P],
                                  in_=pt_ps[:, :, :])

    def emit_groups(ci, gs):
        for g in gs:
            lo = g * OW
            route = ROUTES[ci][g]
            assert route == "D" or lo + OW <= (KD - TAILF32) * P, (
                f"route A group ({ci},{g}) reads uncast vb tail")
            # latency-critical groups run as two independent 512 halves
            # (smaller fill+drain+store links); steady-state groups run
            # 1024 wide (cheaper per byte)
            w = SPLIT_W if (ci, g) in SPLIT_GROUPS else OW
            for piece in range(OW // w):
                o_ps = opool.tile([P, w], F32, name="o_ps", tag="op")
                plo = lo + piece * w
                for hb in range(w // 512):
                    hlo = plo + hb * 512
                    hsl = slice(hlo, hlo + 512)
                    psl = slice(hb * 512, hb * 512 + 512)
                    for h in range(CI // 2):
                        nc.tensor.matmul(
                            o_ps[:, psl],
                            lhsT=pt_sb[:, 2 * h:2 * h + 2, ci * P:(ci + 1) * P],
                            rhs=vb8[:, 2 * h:2 * h + 2, hsl],
                            start=(h == 0),
                            stop=(h == CI // 2 - 1 and route == "D"),
                            perf_mode=mybir.MatmulPerfMode.DoubleRow,
                        )
                    if route == "A":
                        # fold +x on the PE: the drain is a bare ACT copy
                        nc.tensor.matmul(
                            o_ps[:, psl],
                            lhsT=ident[:, :],
                            rhs=vb[:, ci, hsl],
                            start=False,
                            stop=True,
                        )
                gsl = slice(plo, plo + w)
                if route == "D":
                    nc.vector.tensor_add(out=vb[:, ci, gsl], in0=o_ps[:, :],
                                         in1=xf[:, ci, gsl])
                else:
                    nc.scalar.copy(out=vb[:, ci, gsl], in_=o_ps[:, :])
                nc.sync.dma_start(out=out3[:, ci, gsl], in_=vb[:, ci, gsl])

    # Interleaved drain. Emission order IS the static per-engine program
    # order, so: row 0's softmax chain is emitted before phase2 (else the
    # Tile scheduler interleaves rows 1-3 einsum1 into the critical tail),
    # and each row's PT staging tile is emitted right after the previous
    # row's first two groups so the 2-slot PSUM rotation never stalls the
    # row hand-off.
    dsc0 = prologue(0)
    phase2(1)
    emit_tr(1)
    dsc1 = prologue(1)
    emit_pt(0, dsc0)
    emit_groups(0, (0, 1))
    phase2(2)
    emit_tr(2)
    dsc2 = prologue(2)
    phase2(3)
    emit_tr(3)
    # e0's readers (exp0 + all tr reads) are done: PT1 reuses its bank
    emit_pt(1, dsc1, bank=e_ps[0])
    emit_groups(0, (2, 3))
    emit_groups(1, (0, 1))
    dsc3 = prologue(3)
    emit_pt(2, dsc2, bank=e_ps[1])
    emit_groups(1, (2, 3))
    emit_groups(2, (0, 1))
    emit_pt(3, dsc3, bank=e_ps[2])
    emit_groups(2, (2, 3))
    emit_groups(3, (0, 1, 2, 3))


def build():
    nc = bacc.Bacc("TRN2", target_bir_lowering=False, debug=False,
                   num_devices=N_CORES)
    x = nc.dram_tensor("x", [C, S], F32, kind="ExternalInput")
    gamma = nc.dram_tensor("gamma", [1], F32, kind="ExternalInput")
    out = nc.dram_tensor("out", [C, S], BF16, kind="ExternalOutput")
    with tile.TileContext(nc) as tc:
        with ExitStack() as ctx:
            _body(ctx, tc, out.ap(), x.ap(), gamma.ap())
    nc.compile()
    return nc


_NC_CACHE = {}
LAST_RESULTS = None


def kernel(x: np.ndarray, gamma: np.ndarray) -> np.ndarray:
    global LAST_RESULTS
    x = np.ascontiguousarray(np.asarray(x, dtype=np.float32))
    gamma = np.ascontiguousarray(np.asarray(gamma, dtype=np.float32))
    n, c, h, w = x.shape
    assert (n, c, h * w) == (N_CORES, C, S), f"unexpected shape {x.shape}"

    os.environ["BASS_NEVER_TRACE"] = "1"

    if "nc" not in _NC_CACHE:
        _NC_CACHE["nc"] = build()
    nc = _NC_CACHE["nc"]

    in_maps = [
        {"x": x[i].reshape(C, S), "gamma": gamma} for i in range(N_CORES)
    ]
    res = run_bass_kernel_spmd(nc, in_maps, core_ids=list(range(N_CORES)))
    LAST_RESULTS = res
    out = np.stack(
        [np.asarray(res.results[i]["out"]) for i in range(N_CORES)], axis=0
    ).astype(np.float32)
    return out.reshape(n, c, h, w)


if __name__ == "__main__":
    xs = np.random.randn(N_CORES, C, 64, 64).astype(np.float32)
    g = np.zeros((1,), np.float32)
    o = kernel(xs, g)
    print("ok", o.shape, np.abs(o - xs).max())


# revision 32
# speedup vs baseline: 1.1076x; 1.0327x over previous
"""Channel-attention (CAM) Trainium2 Bass kernel.

Reference computation (per batch n):
    v = x[n].reshape(C, S)                 # C=512, S=H*W=4096
    energy = v @ v.T                       # (C, C)
    att = softmax(max_row(energy) - energy, axis=-1)
        = exp(min_row(energy) - energy) / Z
    out[n] = gamma * (att @ v) + x[n]

Sharding: data-parallel over N=8 batches across 8 NeuronCores; each core
computes one full C x C attention locally (no collectives).

v3 design, 53737ns (TimelineSim; prior baseline 61377ns, DMA floor
34.9us). The structural change vs the f32 baseline: the OUTPUT IS
STORED AS BF16 (rel-err ~2.9e-3 at gamma=0, vs the 2e-2 harness gate),
halving store DMA 23.3us -> 11.65us. That makes the back half of the
kernel bound by PSUM->SBUF drain bandwidth instead of DMA: every
einsum2 output group must cross PSUM->SBUF on DVE or ACT (gpsimd has
no PSUM port; DMA and the PE cannot read PSUM), and with the 4 energy
banks resident, only two rotating 4KB PSUM slots remain, so the drain
cycles at (fill + drain)/2 per 1024-wide group (~1.0-1.1us).

Drain routes per 1024-wide group (ROUTES, tuned empirically):
  'D': one DVE tensor_add (f32 PSUM + xf -> bf16 vb), ~1192ns;
  'A': the PE folds +x into the accumulation group with a bf16
       identity matmul (~245ns/512), then one bare ACT copy
       PSUM -> bf16 vb, ~1038ns.
Both engines also carry the fixed softmax work (DVE: row-min from
PSUM, dsc = gamma*I * 1/Z; ACT: exp with Z-accum), balanced via the
*_ENG knobs. Row 0's first group is SPLIT into 512 halves to shorten
the first store's fill+drain+issue chain, and the last group of rows
1-3 draws two 512-wide slots from DEAD energy banks (ET_GROUPS; each
bank's last reader is its row's PT copy), deepening the drain pipeline
exactly where the 2-slot rotation is the binding constraint.

Timeline structure:
  - loads in column stripes; DVE casts x -> bf16 (vb) as stripes land;
    gpsimd casts x -> fp8 (vb8, einsum2 rhs) with a few spilled to ACT;
  - PE transposes chunks into 2 rotating PSUM slots; one 1024-wide
    PSUM->SBUF copy moves each PAIR of chunks to vbT (engines per
    PAIR_ENG); the last TAILF32 chunks transpose from xf in f32
    (no cast dependency) and move as single-chunk copies so the
    load->row0-energy chain stays short;
  - einsum1 runs inline during loads (all 4 rows below KPE, row 0 only
    after), so row 0's energy closes right after the last load; rows
    1-3 close in phase 2 on the PE, overlapped with row 0's softmax;
  - the lower block-triangle is reconstructed via PE transposes as
    early as each row closes (emit_tr), which lets PT staging for rows
    1-3 write into DEAD energy banks instead of taking drain slots;
  - softmax prologues are emitted ahead of the drain; each row's PT
    staging interleaves after the previous row's first two groups.

Remaining gap vs the ~47us ideal (load 23.3 + min-chain ~5 + store
drain ~17): the 2-slot drain cycle and the row-0 energy-close chain;
both are PSUM-capacity-bound (energy rows occupy 4 of 8 banks).
"""

import os
from contextlib import ExitStack, nullcontext as _null

import numpy as np

import concourse.bass as bass
import concourse.tile as tile
from concourse import bacc, mybir
from concourse.bass_utils import run_bass_kernel_spmd
from concourse.masks import make_identity

N_CORES = 8
C = 512
S = 4096
P = 128
CI = C // P  # 4 c-chunks
KD = S // P  # 32 s-chunks of 128
OW = 1024    # einsum2 output group width (1 bf16 PSUM bank)
OG = S // OW  # 4 groups per row

STRIPES = [512] * 7 + [384, 128]
KPE = 18        # chunks whose einsum1 runs all 4 rows inline during loads
TAILF32 = 4     # trailing chunks transposed from xf in f32 (no vb cast dep)
SINGLE_ENG = list("DADA")  # copy engine per tail chunk (KD-TAILF32..KD-1)
PTCOPY_ENG = list("ADAD")  # engine for each row's PT fp8 copy
CAST_ENG = list("DDDDDDDDD")  # bf16-cast engine per stripe
CAST_HP = True
TAIL_PRIO = None
SPLIT_GROUPS = {(0, 0)}  # groups emitted as two 512 halves (latency)
ET_GROUPS = {(1, 3), (2, 3), (3, 3)}  # groups drawing 512 slots from dead energy banks
SPLIT_W = 512
# einsum2 route per (row, group): 'D' = DVE add (PSUM+xf->vb), 'A' = PE
# identity-add + ACT copy. Routes touching the TAILF32 columns (g3) must
# be 'D' -- that region never gets a bf16 cast into vb (asserted below).
ROUTES = (
    ("A", "D", "A", "D"),
    ("A", "D", "A", "D"),
    ("A", "A", "D", "D"),
    ("A", "D", "A", "D"),
)
# engine for each vbT pair copy: 'D' (DVE) / 'A' (ACT)
PAIR_ENG = list("DDAADDAADDAADDAA")
# engine for each of the 36 (stripe, ci) fp8 casts: gpsimd with some ACT
VB8_ENG = list("GGGGGGGGGGGGGGGGAGGGAGGGAGGGAGGGAGGG")

F32 = mybir.dt.float32
BF16 = mybir.dt.bfloat16
FP8 = mybir.dt.float8e4


def _body(ctx: ExitStack, tc: tile.TileContext, out: bass.AP, x: bass.AP,
          gamma: bass.AP):
    nc = tc.nc

    persist = ctx.enter_context(tc.tile_pool(name="persist", bufs=1))
    xf = persist.tile([P, CI, S], F32, name="xf")
    vb = persist.tile([P, CI, S], BF16, name="vb")   # bf16 x, then output
    vb8 = persist.tile([P, CI, S], FP8, name="vb8")
    vbT = persist.tile([P, KD, C], BF16, name="vbT")
    p_sb = persist.tile([P, CI, C], BF16, name="p_sb")
    pt_sb = persist.tile([P, CI, C], FP8, name="pt_sb")
    ident = persist.tile([P, P], BF16, name="ident")
    identf = persist.tile([P, P], F32, name="identf")
    identg = persist.tile([P, P], BF16, name="identg")
    gamma_sb = persist.tile([P, 1], F32, name="gamma_sb")
    mn = persist.tile([P, CI], F32, name="mn")
    zsum = persist.tile([P, CI], F32, name="zsum")
    msc = persist.tile([P, CI], F32, name="msc")

    make_identity(nc, ident)
    make_identity(nc, identf)

    x3 = x.rearrange("(ci p) s -> p ci s", p=P)
    out3 = out.rearrange("(ci p) s -> p ci s", p=P)

    epool = ctx.enter_context(tc.tile_pool(name="epool", bufs=4, space="PSUM"))
    e_ps = [epool.tile([P, C], F32, name=f"e{ci}", tag="et") for ci in range(CI)]
    # 2 rotating 4KB slots shared (by tag) between transpose staging
    # pairs/singles, f32 einsum2 groups and row 0's PT staging tile.
    opool = ctx.enter_context(tc.tile_pool(name="opool", bufs=2, space="PSUM"))
    trpool = ctx.enter_context(tc.tile_pool(name="trp", bufs=3))
    dscpool = ctx.enter_context(tc.tile_pool(name="dscp", bufs=2))

    # ---- load + cast + PE-transpose + inline einsum1 ----
    def emit_e1(k):
        rows = range(CI) if k < KPE else (0,)
        for ci in rows:
            nc.tensor.matmul(
                e_ps[ci][:, ci * P:],
                lhsT=vbT[:, k, ci * P:(ci + 1) * P],
                rhs=vbT[:, k, ci * P:],
                start=(k == 0),
                stop=(k == KD - 1),
            )

    tp_cur = None
    col = 0
    for si, w in enumerate(STRIPES):
        sl = slice(col, col + w)
        for ci in range(CI):
            nc.sync.dma_start(out=xf[:, ci, sl], in_=x3[:, ci, sl])
            cast_hi = min(col + w, (KD - TAILF32) * P)
            if col < cast_hi:
                ceng = nc.scalar if CAST_ENG[si] == "A" else nc.vector
                if CAST_HP:
                    with tc.high_priority():
                        if CAST_ENG[si] == "A":
                            ceng.copy(out=vb[:, ci, col:cast_hi],
                                      in_=xf[:, ci, col:cast_hi])
                        else:
                            ceng.tensor_copy(out=vb[:, ci, col:cast_hi],
                                             in_=xf[:, ci, col:cast_hi])
                elif CAST_ENG[si] == "A":
                    ceng.copy(out=vb[:, ci, col:cast_hi],
                              in_=xf[:, ci, col:cast_hi])
                else:
                    ceng.tensor_copy(out=vb[:, ci, col:cast_hi],
                                     in_=xf[:, ci, col:cast_hi])
            eng = VB8_ENG[si * CI + ci]
            if eng == "A":
                nc.scalar.copy(out=vb8[:, ci, sl], in_=xf[:, ci, sl])
            else:
                nc.gpsimd.tensor_copy(out=vb8[:, ci, sl], in_=xf[:, ci, sl])
        if si == 0:
            nc.sync.dma_start(out=gamma_sb[:, :],
                              in_=gamma.to_broadcast((P, 1)))
            # gamma*I once; dsc = identg * (1/Z) per row
            nc.vector.tensor_scalar(
                out=identg[:, :], in0=ident[:, :], scalar1=gamma_sb[:, :],
                scalar2=None, op0=mybir.AluOpType.mult,
            )
        for k in range(col // P, (col + w) // P):
            tailf = k >= KD - TAILF32
            j, half = k // 2, k % 2
            if tailf:
                # single-chunk staging: 2KB tiles ping-pong through the
                # 2-slot rotation at ~700ns/chunk instead of ~1.9us/pair
                with tc.high_priority(offset=TAIL_PRIO) if TAIL_PRIO else _null():
                    tp_cur = opool.tile([P, C], F32, name="tps", tag="op")
                    for ci in range(CI):
                        nc.tensor.transpose(
                            out=tp_cur[:, ci * P:(ci + 1) * P],
                            in_=xf[:, ci, k * P:(k + 1) * P],
                            identity=identf[:, :],
                        )
                    if SINGLE_ENG[k - (KD - TAILF32)] == "A":
                        nc.scalar.copy(out=vbT[:, k, :], in_=tp_cur[:, :])
                    else:
                        nc.vector.tensor_copy(out=vbT[:, k, :], in_=tp_cur[:, :])
                    emit_e1(k - 2)
                continue
            if half == 0:
                tp_cur = opool.tile([P, 2, C], F32 if tailf else BF16,
                                    name="tp", tag="op")
            for ci in range(CI):
                nc.tensor.transpose(
                    out=tp_cur[:, half, ci * P:(ci + 1) * P],
                    in_=(xf if tailf else vb)[:, ci, k * P:(k + 1) * P],
                    identity=(identf if tailf else ident)[:, :],
                )
            if half == 1:
                # one 1024-wide PSUM->SBUF copy moves the whole pair
                if PAIR_ENG[j] == "D":
                    nc.vector.tensor_copy(out=vbT[:, 2 * j:2 * j + 2, :],
                                          in_=tp_cur[:, :, :])
                else:
                    nc.scalar.copy(out=vbT[:, 2 * j:2 * j + 2, :],
                                   in_=tp_cur[:, :, :])
                # einsum1 for the PREVIOUS pair (its copy landed during
                # this pair's transposes -- the in-order PE never stalls
                # on a copy it just triggered)
                if j > 0:
                    emit_e1(2 * (j - 1))
                    emit_e1(2 * (j - 1) + 1)
        col += w
    emit_e1(KD - 2)
    emit_e1(KD - 1)

    # ---- phase 2 (close rows 1-3), reconstruct, softmax, einsum2 ----
    def phase2(ci):
        for k in range(KPE, KD):
            nc.tensor.matmul(
                e_ps[ci][:, ci * P:],
                lhsT=vbT[:, k, ci * P:(ci + 1) * P],
                rhs=vbT[:, k, ci * P:],
                start=False,
                stop=(k == KD - 1),
            )

    def emit_tr(ci):
        """Lower-triangle reconstruct for row ci (all its upper sources
        are closed). Emitted as early as possible so the source banks die
        early enough for PT staging to reuse them."""
        for cj in range(ci):
            tr_sb = trpool.tile([P, P], F32, name="tr_sb", tag="tr")
            if (ci + cj) % 2 == 0:
                nc.scalar.copy(out=tr_sb[:, :],
                               in_=e_ps[cj][:, ci * P:(ci + 1) * P])
            else:
                nc.vector.tensor_copy(out=tr_sb[:, :],
                                      in_=e_ps[cj][:, ci * P:(ci + 1) * P])
            nc.tensor.matmul(
                e_ps[ci][:, cj * P:(cj + 1) * P],
                lhsT=tr_sb[:, :],
                rhs=identf[:, :],
                is_transpose=True,
                skip_group_check=True,
            )

    def prologue(ci):
        """Row softmax through dsc. Emitted ahead of the drain so
        DVE/ACT always have queued work."""
        nc.vector.tensor_reduce(
            out=mn[:, ci:ci + 1], in_=e_ps[ci][:, :],
            axis=mybir.AxisListType.X, op=mybir.AluOpType.min,
        )
        nc.scalar.activation(
            out=p_sb[:, ci, :], in_=e_ps[ci][:, :],
            func=mybir.ActivationFunctionType.Exp,
            bias=mn[:, ci:ci + 1], scale=-1.0,
            accum_out=zsum[:, ci:ci + 1],
        )
        nc.vector.reciprocal(out=msc[:, ci:ci + 1], in_=zsum[:, ci:ci + 1])
        dsc = dscpool.tile([P, P], BF16, name="dsc", tag="dsc")
        nc.vector.tensor_scalar(
            out=dsc[:, :], in0=identg[:, :], scalar1=msc[:, ci:ci + 1],
            scalar2=None, op0=mybir.AluOpType.mult,
        )
        return dsc

    def emit_pt(ci, dsc, bank=None):
        # PT block = P_block^T @ diag(gamma/Z) (regular matmul; the
        # transpose datapath ignores rhs values so the scale must go
        # through the normal path). PT for rows 1-3 writes into a DEAD
        # energy bank (all its readers ran), keeping the op-slot FIFO
        # free for einsum2 groups; only row 0's PT takes a FIFO turn.
        if bank is None:
            pt_ps = opool.tile([P, CI, P], F32, name="pt_ps", tag="op")
        else:
            pt_ps = bank.rearrange("p (dj q) -> p dj q", dj=CI)
        for dj in range(CI):
            nc.tensor.matmul(
                pt_ps[:, dj, :],
                lhsT=p_sb[:, ci, dj * P:(dj + 1) * P],
                rhs=dsc[:, :],
                skip_group_check=bank is not None,
            )
        if PTCOPY_ENG[ci] == "A":
            nc.scalar.copy(out=pt_sb[:, :, ci * P:(ci + 1) * P],
                           in_=pt_ps[:, :, :])
        else:
            nc.vector.tensor_copy(out=pt_sb[:, :, ci * P:(ci + 1) * P],
                                  in_=pt_ps[:, :, :])

    def emit_groups(ci, gs):
        for g in gs:
            lo = g * OW
            route = ROUTES[ci][g]
            assert route == "D" or lo + OW <= (KD - TAILF32) * P, (
                f"route A group ({ci},{g}) reads uncast vb tail")
            # latency-critical groups run as two independent 512 halves
            # (smaller fill+drain+store links); steady-state groups run
            # 1024 wide (cheaper per byte). ET_GROUPS borrow dead energy
            # banks (released after their PT copies) as extra 512 slots,
            # deepening the drain pipeline at its tail.
            use_et = (ci, g) in ET_GROUPS
            w = SPLIT_W if ((ci, g) in SPLIT_GROUPS or use_et) else OW
            for piece in range(OW // w):
                if use_et:
                    o_ps = epool.tile([P, w], F32, name="o_et", tag="et")
                else:
                    o_ps = opool.tile([P, w], F32, name="o_ps", tag="op")
                plo = lo + piece * w
                for hb in range(w // 512):
                    hlo = plo + hb * 512
                    hsl = slice(hlo, hlo + 512)
                    psl = slice(hb * 512, hb * 512 + 512)
                    for h in range(CI // 2):
                        nc.tensor.matmul(
                            o_ps[:, psl],
                            lhsT=pt_sb[:, 2 * h:2 * h + 2, ci * P:(ci + 1) * P],
                            rhs=vb8[:, 2 * h:2 * h + 2, hsl],
                            start=(h == 0),
                            stop=(h == CI // 2 - 1 and route == "D"),
                            perf_mode=mybir.MatmulPerfMode.DoubleRow,
                        )
                    if route == "A":
                        # fold +x on the PE: the drain is a bare ACT copy
                        nc.tensor.matmul(
                            o_ps[:, psl],
                            lhsT=ident[:, :],
                            rhs=vb[:, ci, hsl],
                            start=False,
                            stop=True,
                        )
                gsl = slice(plo, plo + w)
                if route == "D":
                    nc.vector.tensor_add(out=vb[:, ci, gsl], in0=o_ps[:, :],
                                         in1=xf[:, ci, gsl])
                else:
                    nc.scalar.copy(out=vb[:, ci, gsl], in_=o_ps[:, :])
                nc.sync.dma_start(out=out3[:, ci, gsl], in_=vb[:, ci, gsl])

    # Interleaved drain. Emission order IS the static per-engine program
    # order, so: row 0's softmax chain is emitted before phase2 (else the
    # Tile scheduler interleaves rows 1-3 einsum1 into the critical tail),
    # and each row's PT staging tile is emitted right after the previous
    # row's first two groups so the 2-slot PSUM rotation never stalls the
    # row hand-off.
    dsc0 = prologue(0)
    phase2(1)
    emit_tr(1)
    dsc1 = prologue(1)
    emit_pt(0, dsc0)
    emit_groups(0, (0, 1))
    phase2(2)
    emit_tr(2)
    dsc2 = prologue(2)
    phase2(3)
    emit_tr(3)
    # e0's readers (exp0 + all tr reads) are done: PT1 reuses its bank
    emit_pt(1, dsc1, bank=e_ps[0])
    emit_groups(0, (2, 3))
    emit_groups(1, (0, 1))
    dsc3 = prologue(3)
    emit_pt(2, dsc2, bank=e_ps[1])
    emit_groups(1, (2, 3))
    emit_groups(2, (0, 1))
    emit_pt(3, dsc3, bank=e_ps[2])
    emit_groups(2, (2, 3))
    emit_groups(3, (0, 1, 2, 3))


def build():
    nc = bacc.Bacc("TRN2", target_bir_lowering=False, debug=False,
                   num_devices=N_CORES)
    x = nc.dram_tensor("x", [C, S], F32, kind="ExternalInput")
    gamma = nc.dram_tensor("gamma", [1], F32, kind="ExternalInput")
    out = nc.dram_tensor("out", [C, S], BF16, kind="ExternalOutput")
    with tile.TileContext(nc) as tc:
        with ExitStack() as ctx:
            _body(ctx, tc, out.ap(), x.ap(), gamma.ap())
    nc.compile()
    return nc


_NC_CACHE = {}
LAST_RESULTS = None


def kernel(x: np.ndarray, gamma: np.ndarray) -> np.ndarray:
    global LAST_RESULTS
    x = np.ascontiguousarray(np.asarray(x, dtype=np.float32))
    gamma = np.ascontiguousarray(np.asarray(gamma, dtype=np.float32))
    n, c, h, w = x.shape
    assert (n, c, h * w) == (N_CORES, C, S), f"unexpected shape {x.shape}"

    os.environ["BASS_NEVER_TRACE"] = "1"

    if "nc" not in _NC_CACHE:
        _NC_CACHE["nc"] = build()
    nc = _NC_CACHE["nc"]

    in_maps = [
        {"x": x[i].reshape(C, S), "gamma": gamma} for i in range(N_CORES)
    ]
    res = run_bass_kernel_spmd(nc, in_maps, core_ids=list(range(N_CORES)))
    LAST_RESULTS = res
    out = np.stack(
        [np.asarray(res.results[i]["out"]) for i in range(N_CORES)], axis=0
    ).astype(np.float32)
    return out.reshape(n, c, h, w)


if __name__ == "__main__":
    xs = np.random.randn(N_CORES, C, 64, 64).astype(np.float32)
    g = np.zeros((1,), np.float32)
    o = kernel(xs, g)
    print("ok", o.shape, np.abs(o - xs).max())
